# revision 19
# baseline (speedup 1.0000x reference)
"""Capsule routing kernel v3 (Conv1D k=1 -> dynamic routing) for TRN2, 8 cores.

Data-parallel over batch (8 batches/core), 2 groups of 4 batches stacked on
the 128-partition dim as (b,n).  u_hat is never materialized; routing is
factorized through x:
    s[n,d] = sum_c Q[n,c] W[c,nD+d],  Q = c @ x^T        (PE)
    b[n,t] += sum_c P[c,n] x[c,t],    P = W . vmask      (PE)

v3 changes vs v2 (trace-driven):
  * All transposes via PE identity-matmul (lhsT=chunk, rhs=id128) instead of
    serial DMA XBAR transposes (48.6us -> ~4us).  The softmax partition-sums
    ride the same matmul: rhs = [id128 | on4] gives etT and Z^T in one pass.
  * 1/Z via reciprocal_approx_fast on t-major [128,4] chunks (27us -> ~1us).
  * Softmax normalize fused into the transpose psum->sbuf copy as a
    stride-0-broadcast tensor_mul.
  * Iteration 0 (uniform c) via the same Q matmul path with a constant-1/N
    lhsT (kills 18us of DVE reduce_sum, warms the PE during the x DMA).
  * Q matmul col-tiled: 4 concurrent 32-wide strips (tile_position), rhs
    FD=256 per batch; psum comes out already in q_sb layout (no extraction).
  * x loaded as 8+2 big DMAs split across sync and scalar HWDGE queues.
"""

import contextlib

import numpy as np
import ml_dtypes

import concourse.bass as bass
import concourse.tile as tile
from concourse import bacc, mybir
from concourse.bass_utils import run_bass_kernel_spmd

F32 = mybir.dt.float32
BF16 = mybir.dt.bfloat16
AF = mybir.ActivationFunctionType
AX = mybir.AxisListType
ALU = mybir.AluOpType

B, C, T = 64, 256, 1024
N, D = 32, 16
O = N * D            # 512
NCORES = 8
BPC = B // NCORES    # 8 batches per core
NG = 2               # groups per core
GB = 4               # batches per group (stacked as (b,n) on 128 partitions)
KC = C // 128        # 2 contraction chunks
MT = T // 128        # 8 t-chunks
OG = O // 128        # 4 o-chunks
EPS = 1e-7


def _build_bass():
    nc = bacc.Bacc(
        "TRN2",
        target_bir_lowering=False,
        debug=False,
        enable_asserts=False,
        num_devices=NCORES,
    )
    # x in (c,t) layout: per batch one [128, KC*T] tile (cols = (k,t))
    xb_d = nc.dram_tensor("xb", [BPC, 128, KC * T], BF16, kind="ExternalInput").ap()
    # x transposed: per group one [128, MT*GB*C] tile (cols = (m, b4, c))
    xt_d = nc.dram_tensor("xt", [NG, 128, MT * GB * C], BF16, kind="ExternalInput").ap()
    w_d = nc.dram_tensor("wsb", [KC, 128, O], BF16, kind="ExternalInput").ap()
    wt_d = nc.dram_tensor("wt", [OG, 128, C], BF16, kind="ExternalInput").ap()
    e16_d = nc.dram_tensor("e16", [D, 128], BF16, kind="ExternalInput").ap()
    bm_d = nc.dram_tensor("bm", [OG, 128, 128], BF16, kind="ExternalInput").ap()
    dm4_d = nc.dram_tensor("dm4", [128, O], F32, kind="ExternalInput").ap()
    # identity+ones combined rhs for transpose matmuls: [128, 132]
    idon_d = nc.dram_tensor("idon", [128, 128 + GB], BF16, kind="ExternalInput").ap()
    id128f_d = nc.dram_tensor("id128f", [128, 128], F32, kind="ExternalInput").ap()
    cn_d = nc.dram_tensor("cn", [128, 128], BF16, kind="ExternalInput").ap()
    out_d = nc.dram_tensor("out", [BPC, N, D], F32, kind="ExternalOutput").ap()

    with tile.TileContext(nc) as tc:
        _kernel_body(tc, out_d, xb_d, xt_d, w_d, wt_d, e16_d, bm_d, dm4_d,
                     idon_d, id128f_d, cn_d)
    nc.compile()
    return nc


def _kernel_body(tc, out_d, xb_d, xt_d, w_d, wt_d, e16_d, bm_d, dm4_d,
                 idon_d, id128f_d, cn_d):
    nc = tc.nc
    ctx = contextlib.ExitStack()
    with ctx:
        const = ctx.enter_context(tc.tile_pool(name="const", bufs=1))
        xbp = ctx.enter_context(tc.tile_pool(name="xbp", bufs=BPC))
        xtp = ctx.enter_context(tc.tile_pool(name="xtp", bufs=2 * NG))
        lgp = ctx.enter_context(tc.tile_pool(name="lgp", bufs=NG))
        etp = ctx.enter_context(tc.tile_pool(name="etp", bufs=2))
        ctsp = ctx.enter_context(tc.tile_pool(name="ctsp", bufs=2))
        rzp = ctx.enter_context(tc.tile_pool(name="rzp", bufs=2))
        qsp = ctx.enter_context(tc.tile_pool(name="qsp", bufs=2))
        qtp = ctx.enter_context(tc.tile_pool(name="qtp", bufs=4))
        pbp = ctx.enter_context(tc.tile_pool(name="pbp", bufs=4))
        vtp = ctx.enter_context(tc.tile_pool(name="vtp", bufs=2))
        vbp = ctx.enter_context(tc.tile_pool(name="vbp", bufs=2))
        vmp = ctx.enter_context(tc.tile_pool(name="vmp", bufs=8))
        smp = ctx.enter_context(tc.tile_pool(name="smp", bufs=2))
        vp = ctx.enter_context(tc.tile_pool(name="vp", bufs=4))
        tinyp = ctx.enter_context(tc.tile_pool(name="tinyp", bufs=8))
        # PSUM pools
        ptp = ctx.enter_context(tc.tile_pool(name="ptp", bufs=2, space="PSUM"))
        pqt = ctx.enter_context(tc.tile_pool(name="pqt", bufs=1, space="PSUM"))
        pqp = ctx.enter_context(tc.tile_pool(name="pqp", bufs=2, space="PSUM"))
        pband = ctx.enter_context(tc.tile_pool(name="pband", bufs=3, space="PSUM"))

        # --- constants (sync queue) ---
        w_sb = [const.tile([128, O], BF16, name=f"w{k}", tag=f"w{k}") for k in range(KC)]
        for k in range(KC):
            nc.sync.dma_start(w_sb[k][:], w_d[k])
        wt_sb = [const.tile([128, C], BF16, name=f"wt{g}", tag=f"wt{g}") for g in range(OG)]
        for g in range(OG):
            nc.sync.dma_start(wt_sb[g][:], wt_d[g])
        e16 = const.tile([D, 128], BF16, name="e16", tag="e16")
        nc.sync.dma_start(e16[:], e16_d[:])
        bm = [const.tile([128, 128], BF16, name=f"bm{g}", tag=f"bm{g}") for g in range(OG)]
        for g in range(OG):
            nc.sync.dma_start(bm[g][:], bm_d[g])
        dm4 = const.tile([128, O], F32, name="dm4", tag="dm4")
        nc.sync.dma_start(dm4[:], dm4_d[:])
        idon = const.tile([128, 128 + GB], BF16, name="idon", tag="idon")
        nc.sync.dma_start(idon[:], idon_d[:])
        id128f = const.tile([128, 128], F32, name="id128f", tag="id128f")
        nc.sync.dma_start(id128f[:], id128f_d[:])
        cn = const.tile([128, 128], BF16, name="cn", tag="cn")
        nc.sync.dma_start(cn[:], cn_d[:])

        # --- x loads: xt on scalar queue (needed first, split in half for
        # earlier iter-0 start), xb on sync queue ---
        HM = MT // 2
        xth = {}
        for g in range(NG):
            for h in range(2):
                t = xtp.tile([128, HM * GB * C], BF16, name="xt", tag="xt")
                nc.scalar.dma_start(
                    t[:], xt_d[g][:, h * HM * GB * C:(h + 1) * HM * GB * C])
                xth[g, h] = t
        xb = {}
        for b in range(BPC):
            t = xbp.tile([128, KC * T], BF16, name="xb", tag="xb")
            nc.sync.dma_start(t[:], xb_d[b])
            xb[b] = t

        logits = {g: lgp.tile([128, T], F32, name="lg", tag="lg") for g in range(NG)}
        ets = {}

        # scratch for activation-table preloading (keeps the ~1.3us
        # ACT_TABLE_LOAD for Exp<->Sqrt switches off the critical path)
        scr = const.tile([128, 1], F32, name="scr", tag="scr")
        scr2 = const.tile([128, 1], F32, name="scr2", tag="scr2")
        nc.gpsimd.memset(scr[:], 1.0)

        def preload(func, dep=None):
            nc.scalar.activation(scr2[:], scr[:] if dep is None else dep, func)

        def softmax_transpose(g):
            """logits [bn, T] -> cts [128t, (m, bn)] bf16, normalized.

            Per 128-t chunk: one PE matmul with lhsT = exp-chunk and
            rhs = [id128 | on4] yields etT (cols 0:128) and Z^T (cols 128:132)
            in psum; 1/Z via fast reciprocal; normalize fused into the
            psum->sbuf copy as a broadcast multiply.
            """
            lg = logits[g]
            et = etp.tile([128, T], BF16, name="et", tag="et")
            ets[g] = et
            nc.scalar.activation(et[:], lg[:], AF.Exp)
            cts = ctsp.tile([128, MT * 128], BF16, name="cts", tag="cts")
            rzt = rzp.tile([128, MT * GB], F32, name="rzt", tag="rzt")
            zt = rzp.tile([128, MT * GB], F32, name="zt", tag="zt")
            tps = []
            for m in range(MT):
                tp = ptp.tile([128, 512], F32, name="tp", tag="tp")
                nc.tensor.matmul(
                    tp[:, 0:128 + GB], et[:, m * 128:(m + 1) * 128], idon[:],
                    start=True, stop=True,
                )
                tps.append(tp)
                nc.vector.tensor_copy(
                    zt[:, m * GB:(m + 1) * GB], tp[:, 128:128 + GB],
                )
                if m % 2 == 1:
                    # batched reciprocal for 2 chunks at once
                    nc.vector.reciprocal(
                        rzt[:, (m - 1) * GB:(m + 1) * GB],
                        zt[:, (m - 1) * GB:(m + 1) * GB],
                    )
                    for mm in range(m - 1, m + 1):
                        nc.vector.tensor_tensor(
                            cts[:, mm * 128:(mm + 1) * 128].rearrange(
                                "p (b n) -> p b n", n=N),
                            tps[mm][:, 0:128].rearrange("p (b n) -> p b n", n=N),
                            rzt[:, mm * GB:(mm + 1) * GB].unsqueeze(2).broadcast_to(
                                [128, GB, N]),
                            ALU.mult,
                        )
            return cts

        def q_phase(g, cts_ap, cstride):
            """Q[bn, c] col-tiled: strip b4 <- cts chunk-strip ^T @ xt4-slice.

            cts_ap: SBUF AP base; strip (m, b4) slice is
            cts_ap[:, m*cstride + 32*b4 : m*cstride + 32*b4 + 32]
            (cstride=0 for the constant iteration-0 lhsT).
            """
            # Two psum banks: strips {0,1} in qA, {2,3} in qB.  One open
            # accumulation group per bank; pairs (0,2) then (1,3) issue
            # back-to-back at different PE col-groups -> concurrent.
            qA = pqp.tile([128, 512], F32, name="qA", tag="q_ps")
            qB = pqp.tile([128, 512], F32, name="qB", tag="q_ps")
            for phase in range(2):
                for m in range(MT):
                    for b4 in (phase, phase + 2):
                        q_ps = qA if b4 < 2 else qB
                        off = m * cstride + 32 * b4
                        xt_t = xth[g, m // HM]
                        mo = (m % HM) * GB * C + b4 * C
                        nc.tensor.matmul(
                            q_ps[32 * b4:32 * (b4 + 1), 0:C],
                            cts_ap[:, off:off + 32],
                            xt_t[:, mo:mo + C],
                            start=(m == 0), stop=(m == MT - 1),
                            tile_position=(0, 32 * b4),
                        )
            q_sb = qsp.tile([128, C], BF16, name="q_sb", tag="q_sb")
            nc.scalar.copy(q_sb[0:64, :], qA[0:64, 0:C])
            nc.scalar.copy(q_sb[64:128, :], qB[64:128, 0:C])
            # transpose Q via identity matmul, then s = Qt^T @ W
            s_ps = pband.tile([128, O], F32, name="s_ps", tag="band")
            qts = []
            for h in range(KC):
                qt_ps = pqt.tile([128, 512], F32, name="qt_ps", tag="pqt")
                nc.tensor.matmul(
                    qt_ps[:, 0:128], q_sb[:, h * 128:(h + 1) * 128], idon[:, 0:128],
                    start=True, stop=True,
                )
                qt = qtp.tile([128, 128], BF16, name="qt", tag="qt")
                nc.scalar.copy(qt[:], qt_ps[:, 0:128])
                qts.append(qt)
            for h in range(KC):
                nc.tensor.matmul(
                    s_ps[:], qts[h][:], w_sb[h][:],
                    start=(h == 0), stop=(h == KC - 1),
                )
            return s_ps

        def extract_squash(s_ps):
            """psum s_full (128(b,n), O) -> v (128, D) f32 via mask+strided reduce."""
            sm = smp.tile([128, O], F32, name="sm", tag="sm")
            nc.vector.tensor_mul(sm[:], s_ps[:], dm4[:])
            s_t = tinyp.tile([128, D], F32, name="s_t", tag="s_t")
            nc.vector.reduce_sum(
                s_t[:], sm[:].rearrange("p (n d) -> p d n", d=D), axis=AX.X
            )
            sq = tinyp.tile([128, D], F32, name="sq", tag="sq")
            nc.vector.tensor_mul(sq[:], s_t[:], s_t[:])
            s2 = tinyp.tile([128, 1], F32, name="s2", tag="s2")
            nc.vector.reduce_sum(s2[:], sq[:], axis=AX.X)
            s2e = tinyp.tile([128, 1], F32, name="s2e", tag="s2e")
            nc.vector.tensor_scalar_add(s2e[:], s2[:], EPS)
            rt = tinyp.tile([128, 1], F32, name="rt", tag="rt")
            nc.scalar.sqrt(rt[:], s2e[:])
            d1 = tinyp.tile([128, 1], F32, name="d1", tag="d1")
            nc.vector.tensor_scalar_add(d1[:], s2e[:], 1.0)
            r1 = tinyp.tile([128, 1], F32, name="r1", tag="r1")
            nc.vector.reciprocal(r1[:], d1[:])
            sc = tinyp.tile([128, 1], F32, name="sc", tag="sc")
            nc.vector.tensor_mul(sc[:], rt[:], r1[:])
            v = vp.tile([128, D], F32, name="v", tag="v")
            nc.vector.tensor_scalar_mul(v[:], s_t[:], sc[:])
            return v

        def update(g, v, first):
            """logits ((b,n), t) += x^T (W . vmask) for the 4 stacked batches."""
            vt_ps = pqt.tile([128, 512], F32, name="vt_ps", tag="pqt")
            nc.tensor.transpose(vt_ps[0:D, 0:128], v[:], id128f[:])
            vt_bf = vtp.tile([D, 128], BF16, name="vt_bf", tag="vt_bf")
            nc.vector.tensor_copy(vt_bf[:], vt_ps[0:D, 0:128])
            vbc_ps = pqt.tile([128, 512], F32, name="vbc", tag="pqt")
            nc.tensor.matmul(vbc_ps[:, 0:128], e16[:], vt_bf[:], start=True, stop=True)
            vbc_sb = vbp.tile([128, 128], BF16, name="vbc_sb", tag="vbc_sb")
            nc.scalar.copy(vbc_sb[:], vbc_ps[:, 0:128])
            vms = []
            for g4 in range(OG):
                vm = vmp.tile([128, 128], BF16, name="vm", tag="vm")
                nc.vector.tensor_mul(vm[:], vbc_sb[:], bm[g4][:])
                vms.append(vm)
            p_sb = []
            for h in range(KC):
                p_ps = pqt.tile([128, 512], F32, name="p_ps", tag="pqt")
                for g4 in range(OG):
                    nc.tensor.matmul(
                        p_ps[:, 0:128], wt_sb[g4][:, h * 128:(h + 1) * 128], vms[g4][:],
                        start=(g4 == 0), stop=(g4 == OG - 1),
                    )
                pb = pbp.tile([128, 128], BF16, name="pb", tag="pb")
                nc.scalar.copy(pb[:], p_ps[:, 0:128])
                p_sb.append(pb)
            lg = logits[g]
            # two banks (t-halves); strip pairing offset by 1 so concurrent
            # MMs land on different PE col-groups
            a_ps = [
                pband.tile([128, 512], F32, name=f"a_ps{j}", tag="band")
                for j in range(2)
            ]
            for step in range(GB):
                for k in range(KC):
                    for j in range(2):
                        b4 = (step + j) % GB
                        b = g * GB + b4
                        nc.tensor.matmul(
                            a_ps[j][32 * b4:32 * (b4 + 1), :],
                            p_sb[k][:, 32 * b4:32 * (b4 + 1)],
                            xb[b][:, k * T + j * 512:k * T + j * 512 + 512],
                            start=(k == 0), stop=(k == KC - 1),
                            tile_position=(0, 32 * b4),
                        )
            for j in range(2):
                if first:
                    nc.scalar.copy(lg[:, j * 512:(j + 1) * 512], a_ps[j][:])
                else:
                    nc.vector.tensor_add(
                        lg[:, j * 512:(j + 1) * 512],
                        lg[:, j * 512:(j + 1) * 512], a_ps[j][:],
                    )

        # --- iteration 0 (uniform c = 1/N via constant lhsT) ---
        preload(AF.Sqrt)
        sps = {g: q_phase(g, cn[:], 0) for g in range(NG)}
        vs = {g: extract_squash(sps[g]) for g in range(NG)}
        preload(AF.Exp, dep=vs[1][:, 0:1])
        for g in range(NG):
            update(g, vs[g], first=True)

        # --- iterations 1, 2 ---
        for it in (1, 2):
            ctss = {g: softmax_transpose(g) for g in range(NG)}
            preload(AF.Sqrt, dep=ets[1][:, 0:1])
            sps = {g: q_phase(g, ctss[g][:], 128) for g in range(NG)}
            for g in range(NG):
                vs[g] = extract_squash(sps[g])
            if it == 1:
                preload(AF.Exp, dep=vs[1][:, 0:1])
                for g in range(NG):
                    update(g, vs[g], first=False)
            else:
                for g in range(NG):
                    nc.sync.dma_start(
                        out_d[g * GB:(g + 1) * GB], vs[g][:],
                    )


_NC_CACHE = {}


def _get_nc():
    if "nc" not in _NC_CACHE:
        _NC_CACHE["nc"] = _build_bass()
    return _NC_CACHE["nc"]


def _make_in_maps(x, W):
    BFnp = ml_dtypes.bfloat16
    x = np.asarray(x, np.float32)
    W = np.asarray(W, np.float32)
    w_bf = np.ascontiguousarray(W.reshape(KC, 128, O)).astype(BFnp)
    wt = np.ascontiguousarray(W.reshape(C, OG, 128).transpose(1, 2, 0)).astype(BFnp)
    e16 = (np.arange(128)[None, :] % D == np.arange(D)[:, None]).astype(BFnp)
    oo = np.arange(128)
    bn = np.arange(128)
    bm = np.stack(
        [((g * 8 + oo[:, None] // D) == (bn[None, :] % N)) for g in range(OG)]
    ).astype(BFnp)
    dm4 = ((np.arange(O)[None, :] // D) == (bn[:, None] % N)).astype(np.float32)
    # [id128 | on4]: on4[bn, j] = (bn // N == j)
    idon = np.zeros((128, 128 + GB), np.float32)
    idon[:, :128] = np.eye(128)
    idon[bn, 128 + bn // N] = 1.0
    idon = idon.astype(BFnp)
    id128f = np.eye(128, dtype=np.float32)
    cn = np.full((128, 128), 1.0 / N, BFnp)

    in_maps = []
    for core in range(NCORES):
        xs = x[core * BPC:(core + 1) * BPC]              # (8, C, T)
        # (b, c, t) -> [b, 128, (k, t)]
        xbt = np.ascontiguousarray(
            xs.reshape(BPC, KC, 128, T).transpose(0, 2, 1, 3).reshape(
                BPC, 128, KC * T)
        ).astype(BFnp)
        # transposed layout: [g, 128t, (m, b4, c)]
        xt4 = np.zeros((NG, 128, MT * GB * C), BFnp)
        for g in range(NG):
            for b4 in range(GB):
                xtb = xs[g * GB + b4].T                  # (T, C) f32
                blocks = xtb.reshape(MT, 128, C).astype(BFnp)  # (m, tl, c)
                for m in range(MT):
                    xt4[g, :, m * GB * C + b4 * C:(m * GB + b4 + 1) * C] = blocks[m]
        in_maps.append(
            {
                "xb": xbt, "xt": xt4, "wsb": w_bf, "wt": wt, "e16": e16,
                "bm": bm, "dm4": dm4, "idon": idon, "id128f": id128f,
                "cn": cn,
            }
        )
    return in_maps


def run(x, W, trace=False):
    in_maps = _make_in_maps(x, W)
    nc = _get_nc()
    res = run_bass_kernel_spmd(nc, in_maps, core_ids=list(range(NCORES)), trace=trace)
    out = np.concatenate([r["out"] for r in res.results], axis=0)
    return out, res


def kernel(x, W, out_num_capsule=N, out_dim_capsule=D, routings=3, **_):
    out, _res = run(x, W, trace=False)
    return out


# revision 20
# speedup vs baseline: 1.0496x; 1.0496x over previous
"""Capsule routing kernel v3 (Conv1D k=1 -> dynamic routing) for TRN2, 8 cores.

Data-parallel over batch (8 batches/core), 2 groups of 4 batches stacked on
the 128-partition dim as (b,n).  u_hat is never materialized; routing is
factorized through x:
    s[n,d] = sum_c Q[n,c] W[c,nD+d],  Q = c @ x^T        (PE)
    b[n,t] += sum_c P[c,n] x[c,t],    P = W . vmask      (PE)

v3 changes vs v2 (trace-driven):
  * All transposes via PE identity-matmul (lhsT=chunk, rhs=id128) instead of
    serial DMA XBAR transposes (48.6us -> ~4us).  The softmax partition-sums
    ride the same matmul: rhs = [id128 | on4] gives etT and Z^T in one pass.
  * 1/Z via reciprocal_approx_fast on t-major [128,4] chunks (27us -> ~1us).
  * Softmax normalize fused into the transpose psum->sbuf copy as a
    stride-0-broadcast tensor_mul.
  * Iteration 0 (uniform c) via the same Q matmul path with a constant-1/N
    lhsT (kills 18us of DVE reduce_sum, warms the PE during the x DMA).
  * Q matmul col-tiled: 4 concurrent 32-wide strips (tile_position), rhs
    FD=256 per batch; psum comes out already in q_sb layout (no extraction).
  * x loaded as 8+2 big DMAs split across sync and scalar HWDGE queues.
"""

import contextlib

import numpy as np
import ml_dtypes

import concourse.bass as bass
import concourse.tile as tile
from concourse import bacc, mybir
from concourse.bass_utils import run_bass_kernel_spmd

F32 = mybir.dt.float32
BF16 = mybir.dt.bfloat16
AF = mybir.ActivationFunctionType
AX = mybir.AxisListType
ALU = mybir.AluOpType

B, C, T = 64, 256, 1024
N, D = 32, 16
O = N * D            # 512
NCORES = 8
BPC = B // NCORES    # 8 batches per core
NG = 2               # groups per core
GB = 4               # batches per group (stacked as (b,n) on 128 partitions)
KC = C // 128        # 2 contraction chunks
MT = T // 128        # 8 t-chunks
OG = O // 128        # 4 o-chunks
EPS = 1e-7


def _build_bass():
    nc = bacc.Bacc(
        "TRN2",
        target_bir_lowering=False,
        debug=False,
        enable_asserts=False,
        num_devices=NCORES,
    )
    # x in (c,t) layout: per batch one [128, KC*T] tile (cols = (k,t))
    xb_d = nc.dram_tensor("xb", [BPC, 128, KC * T], BF16, kind="ExternalInput").ap()
    # x transposed: per group one [128, MT*GB*C] tile (cols = (m, b4, c))
    xt_d = nc.dram_tensor("xt", [NG, 128, MT * GB * C], BF16, kind="ExternalInput").ap()
    w_d = nc.dram_tensor("wsb", [KC, 128, O], BF16, kind="ExternalInput").ap()
    wt_d = nc.dram_tensor("wt", [OG, 128, C], BF16, kind="ExternalInput").ap()
    e16_d = nc.dram_tensor("e16", [D, 128], BF16, kind="ExternalInput").ap()
    bm_d = nc.dram_tensor("bm", [OG, 128, 128], BF16, kind="ExternalInput").ap()
    dm4_d = nc.dram_tensor("dm4", [128, O], F32, kind="ExternalInput").ap()
    # identity+ones combined rhs for transpose matmuls: [128, 132]
    idon_d = nc.dram_tensor("idon", [128, 128 + GB], BF16, kind="ExternalInput").ap()
    id128f_d = nc.dram_tensor("id128f", [128, 128], F32, kind="ExternalInput").ap()
    cn_d = nc.dram_tensor("cn", [128, 128], BF16, kind="ExternalInput").ap()
    out_d = nc.dram_tensor("out", [BPC, N, D], F32, kind="ExternalOutput").ap()

    with tile.TileContext(nc) as tc:
        _kernel_body(tc, out_d, xb_d, xt_d, w_d, wt_d, e16_d, bm_d, dm4_d,
                     idon_d, id128f_d, cn_d)
    nc.compile()
    return nc


def _kernel_body(tc, out_d, xb_d, xt_d, w_d, wt_d, e16_d, bm_d, dm4_d,
                 idon_d, id128f_d, cn_d):
    nc = tc.nc
    ctx = contextlib.ExitStack()
    with ctx:
        const = ctx.enter_context(tc.tile_pool(name="const", bufs=1))
        xbp = ctx.enter_context(tc.tile_pool(name="xbp", bufs=BPC))
        xtp = ctx.enter_context(tc.tile_pool(name="xtp", bufs=2 * NG))
        lgp = ctx.enter_context(tc.tile_pool(name="lgp", bufs=NG))
        etp = ctx.enter_context(tc.tile_pool(name="etp", bufs=2))
        ctsp = ctx.enter_context(tc.tile_pool(name="ctsp", bufs=2))
        rzp = ctx.enter_context(tc.tile_pool(name="rzp", bufs=2))
        qsp = ctx.enter_context(tc.tile_pool(name="qsp", bufs=2))
        qtp = ctx.enter_context(tc.tile_pool(name="qtp", bufs=4))
        pbp = ctx.enter_context(tc.tile_pool(name="pbp", bufs=4))
        vtp = ctx.enter_context(tc.tile_pool(name="vtp", bufs=2))
        vbp = ctx.enter_context(tc.tile_pool(name="vbp", bufs=2))
        vmp = ctx.enter_context(tc.tile_pool(name="vmp", bufs=8))
        smp = ctx.enter_context(tc.tile_pool(name="smp", bufs=2))
        vp = ctx.enter_context(tc.tile_pool(name="vp", bufs=4))
        tinyp = ctx.enter_context(tc.tile_pool(name="tinyp", bufs=8))
        # PSUM pools
        ptp = ctx.enter_context(tc.tile_pool(name="ptp", bufs=2, space="PSUM"))
        pqt = ctx.enter_context(tc.tile_pool(name="pqt", bufs=1, space="PSUM"))
        pqp = ctx.enter_context(tc.tile_pool(name="pqp", bufs=2, space="PSUM"))
        pband = ctx.enter_context(tc.tile_pool(name="pband", bufs=3, space="PSUM"))

        # --- constants (sync queue) ---
        w_sb = [const.tile([128, O], BF16, name=f"w{k}", tag=f"w{k}") for k in range(KC)]
        for k in range(KC):
            nc.sync.dma_start(w_sb[k][:], w_d[k])
        wt_sb = [const.tile([128, C], BF16, name=f"wt{g}", tag=f"wt{g}") for g in range(OG)]
        for g in range(OG):
            nc.sync.dma_start(wt_sb[g][:], wt_d[g])
        e16 = const.tile([D, 128], BF16, name="e16", tag="e16")
        nc.sync.dma_start(e16[:], e16_d[:])
        bm = [const.tile([128, 128], BF16, name=f"bm{g}", tag=f"bm{g}") for g in range(OG)]
        for g in range(OG):
            nc.sync.dma_start(bm[g][:], bm_d[g])
        dm4 = const.tile([128, O], F32, name="dm4", tag="dm4")
        nc.sync.dma_start(dm4[:], dm4_d[:])
        idon = const.tile([128, 128 + GB], BF16, name="idon", tag="idon")
        nc.sync.dma_start(idon[:], idon_d[:])
        id128f = const.tile([128, 128], F32, name="id128f", tag="id128f")
        nc.sync.dma_start(id128f[:], id128f_d[:])
        cn = const.tile([128, 128], BF16, name="cn", tag="cn")
        nc.sync.dma_start(cn[:], cn_d[:])

        # --- x loads: xt on scalar queue (needed first, split in half for
        # earlier iter-0 start), xb on sync queue ---
        HM = MT // 2
        xth = {}
        for g in range(NG):
            for h in range(2):
                t = xtp.tile([128, HM * GB * C], BF16, name="xt", tag="xt")
                nc.scalar.dma_start(
                    t[:], xt_d[g][:, h * HM * GB * C:(h + 1) * HM * GB * C])
                xth[g, h] = t
        xb = {}
        for b in range(BPC):
            t = xbp.tile([128, KC * T], BF16, name="xb", tag="xb")
            nc.sync.dma_start(t[:], xb_d[b])
            xb[b] = t

        logits = {g: lgp.tile([128, T], F32, name="lg", tag="lg") for g in range(NG)}
        ets = {}

        # scratch for activation-table preloading (keeps the ~1.3us
        # ACT_TABLE_LOAD for Exp<->Sqrt switches off the critical path)
        scr = const.tile([128, 1], F32, name="scr", tag="scr")
        scr2 = const.tile([128, 1], F32, name="scr2", tag="scr2")
        nc.gpsimd.memset(scr[:], 1.0)

        def preload(func, dep=None):
            nc.scalar.activation(scr2[:], scr[:] if dep is None else dep, func)

        def softmax_transpose(g):
            """logits [bn, T] -> cts [128t, (m, bn)] bf16, normalized.

            Per 128-t chunk: one PE matmul with lhsT = exp-chunk and
            rhs = [id128 | on4] yields etT (cols 0:128) and Z^T (cols 128:132)
            in psum; 1/Z via fast reciprocal; normalize fused into the
            psum->sbuf copy as a broadcast multiply.
            """
            lg = logits[g]
            et = etp.tile([128, T], BF16, name="et", tag="et")
            ets[g] = et
            nc.scalar.activation(et[:], lg[:], AF.Exp)
            cts = ctsp.tile([128, MT * 128], BF16, name="cts", tag="cts")
            rzt = rzp.tile([128, MT * GB], F32, name="rzt", tag="rzt")
            for m in range(MT):
                tp = ptp.tile([128, 512], F32, name="tp", tag="tp")
                nc.tensor.matmul(
                    tp[:, 0:128 + GB], et[:, m * 128:(m + 1) * 128], idon[:],
                    start=True, stop=True,
                )
                nc.vector.reciprocal(
                    rzt[:, m * GB:(m + 1) * GB], tp[:, 128:128 + GB],
                )
                # cts[:, m-chunk] = etT * (1/Z) broadcast over the 32 n-cols
                nc.vector.tensor_tensor(
                    cts[:, m * 128:(m + 1) * 128].rearrange(
                        "p (b n) -> p b n", n=N),
                    tp[:, 0:128].rearrange("p (b n) -> p b n", n=N),
                    rzt[:, m * GB:(m + 1) * GB].unsqueeze(2).broadcast_to(
                        [128, GB, N]),
                    ALU.mult,
                )
            return cts

        def q_phase(g, cts_ap, cstride):
            """Q[bn, c] col-tiled: strip b4 <- cts chunk-strip ^T @ xt4-slice.

            cts_ap: SBUF AP base; strip (m, b4) slice is
            cts_ap[:, m*cstride + 32*b4 : m*cstride + 32*b4 + 32]
            (cstride=0 for the constant iteration-0 lhsT).
            """
            # Two psum banks: strips {0,1} in qA, {2,3} in qB.  One open
            # accumulation group per bank; pairs (0,2) then (1,3) issue
            # back-to-back at different PE col-groups -> concurrent.
            qA = pqp.tile([128, 512], F32, name="qA", tag="q_ps")
            qB = pqp.tile([128, 512], F32, name="qB", tag="q_ps")
            for phase in range(2):
                for m in range(MT):
                    for b4 in (phase, phase + 2):
                        q_ps = qA if b4 < 2 else qB
                        off = m * cstride + 32 * b4
                        xt_t = xth[g, m // HM]
                        mo = (m % HM) * GB * C + b4 * C
                        nc.tensor.matmul(
                            q_ps[32 * b4:32 * (b4 + 1), 0:C],
                            cts_ap[:, off:off + 32],
                            xt_t[:, mo:mo + C],
                            start=(m == 0), stop=(m == MT - 1),
                            tile_position=(0, 32 * b4),
                        )
            q_sb = qsp.tile([128, C], BF16, name="q_sb", tag="q_sb")
            nc.scalar.copy(q_sb[0:64, :], qA[0:64, 0:C])
            nc.scalar.copy(q_sb[64:128, :], qB[64:128, 0:C])
            # transpose Q via identity matmul, then s = Qt^T @ W
            s_ps = pband.tile([128, O], F32, name="s_ps", tag="band")
            qts = []
            for h in range(KC):
                qt_ps = pqt.tile([128, 512], F32, name="qt_ps", tag="pqt")
                nc.tensor.matmul(
                    qt_ps[:, 0:128], q_sb[:, h * 128:(h + 1) * 128], idon[:, 0:128],
                    start=True, stop=True,
                )
                qt = qtp.tile([128, 128], BF16, name="qt", tag="qt")
                nc.scalar.copy(qt[:], qt_ps[:, 0:128])
                qts.append(qt)
            for h in range(KC):
                nc.tensor.matmul(
                    s_ps[:], qts[h][:], w_sb[h][:],
                    start=(h == 0), stop=(h == KC - 1),
                )
            return s_ps

        def extract_squash(s_ps):
            """psum s_full (128(b,n), O) -> v (128, D) f32 via mask+strided reduce."""
            sm = smp.tile([128, O], F32, name="sm", tag="sm")
            nc.vector.tensor_mul(sm[:], s_ps[:], dm4[:])
            s_t = tinyp.tile([128, D], F32, name="s_t", tag="s_t")
            nc.vector.reduce_sum(
                s_t[:], sm[:].rearrange("p (n d) -> p d n", d=D), axis=AX.X
            )
            sq = tinyp.tile([128, D], F32, name="sq", tag="sq")
            nc.vector.tensor_mul(sq[:], s_t[:], s_t[:])
            s2 = tinyp.tile([128, 1], F32, name="s2", tag="s2")
            nc.vector.reduce_sum(s2[:], sq[:], axis=AX.X)
            s2e = tinyp.tile([128, 1], F32, name="s2e", tag="s2e")
            nc.vector.tensor_scalar_add(s2e[:], s2[:], EPS)
            rt = tinyp.tile([128, 1], F32, name="rt", tag="rt")
            nc.scalar.sqrt(rt[:], s2e[:])
            d1 = tinyp.tile([128, 1], F32, name="d1", tag="d1")
            nc.vector.tensor_scalar_add(d1[:], s2e[:], 1.0)
            r1 = tinyp.tile([128, 1], F32, name="r1", tag="r1")
            nc.vector.reciprocal(r1[:], d1[:])
            sc = tinyp.tile([128, 1], F32, name="sc", tag="sc")
            nc.vector.tensor_mul(sc[:], rt[:], r1[:])
            v = vp.tile([128, D], F32, name="v", tag="v")
            nc.vector.tensor_scalar_mul(v[:], s_t[:], sc[:])
            return v

        def update(g, v, first):
            """logits ((b,n), t) += x^T (W . vmask) for the 4 stacked batches."""
            vt_ps = pqt.tile([128, 512], F32, name="vt_ps", tag="pqt")
            nc.tensor.transpose(vt_ps[0:D, 0:128], v[:], id128f[:])
            vt_bf = vtp.tile([D, 128], BF16, name="vt_bf", tag="vt_bf")
            nc.vector.tensor_copy(vt_bf[:], vt_ps[0:D, 0:128])
            vbc_ps = pqt.tile([128, 512], F32, name="vbc", tag="pqt")
            nc.tensor.matmul(vbc_ps[:, 0:128], e16[:], vt_bf[:], start=True, stop=True)
            vbc_sb = vbp.tile([128, 128], BF16, name="vbc_sb", tag="vbc_sb")
            nc.scalar.copy(vbc_sb[:], vbc_ps[:, 0:128])
            vms = []
            for g4 in range(OG):
                vm = vmp.tile([128, 128], BF16, name="vm", tag="vm")
                nc.vector.tensor_mul(vm[:], vbc_sb[:], bm[g4][:])
                vms.append(vm)
            p_sb = []
            for h in range(KC):
                p_ps = pqt.tile([128, 512], F32, name="p_ps", tag="pqt")
                for g4 in range(OG):
                    nc.tensor.matmul(
                        p_ps[:, 0:128], wt_sb[g4][:, h * 128:(h + 1) * 128], vms[g4][:],
                        start=(g4 == 0), stop=(g4 == OG - 1),
                    )
                pb = pbp.tile([128, 128], BF16, name="pb", tag="pb")
                nc.scalar.copy(pb[:], p_ps[:, 0:128])
                p_sb.append(pb)
            lg = logits[g]
            # two banks (t-halves); strip pairing offset by 1 so concurrent
            # MMs land on different PE col-groups
            a_ps = [
                pband.tile([128, 512], F32, name=f"a_ps{j}", tag="band")
                for j in range(2)
            ]
            for step in range(GB):
                for k in range(KC):
                    for j in range(2):
                        b4 = (step + j) % GB
                        b = g * GB + b4
                        nc.tensor.matmul(
                            a_ps[j][32 * b4:32 * (b4 + 1), :],
                            p_sb[k][:, 32 * b4:32 * (b4 + 1)],
                            xb[b][:, k * T + j * 512:k * T + j * 512 + 512],
                            start=(k == 0), stop=(k == KC - 1),
                            tile_position=(0, 32 * b4),
                        )
            for j in range(2):
                if first:
                    nc.scalar.copy(lg[:, j * 512:(j + 1) * 512], a_ps[j][:])
                else:
                    nc.vector.tensor_add(
                        lg[:, j * 512:(j + 1) * 512],
                        lg[:, j * 512:(j + 1) * 512], a_ps[j][:],
                    )

        # --- iteration 0 (uniform c = 1/N via constant lhsT) ---
        preload(AF.Sqrt)
        sps = {g: q_phase(g, cn[:], 0) for g in range(NG)}
        vs = {g: extract_squash(sps[g]) for g in range(NG)}
        preload(AF.Exp, dep=vs[1][:, 0:1])
        for g in range(NG):
            update(g, vs[g], first=True)

        # --- iterations 1, 2 ---
        for it in (1, 2):
            ctss = {g: softmax_transpose(g) for g in range(NG)}
            preload(AF.Sqrt, dep=ets[1][:, 0:1])
            sps = {g: q_phase(g, ctss[g][:], 128) for g in range(NG)}
            for g in range(NG):
                vs[g] = extract_squash(sps[g])
            if it == 1:
                preload(AF.Exp, dep=vs[1][:, 0:1])
                for g in range(NG):
                    update(g, vs[g], first=False)
            else:
                for g in range(NG):
                    nc.sync.dma_start(
                        out_d[g * GB:(g + 1) * GB], vs[g][:],
                    )


_NC_CACHE = {}


def _get_nc():
    if "nc" not in _NC_CACHE:
        _NC_CACHE["nc"] = _build_bass()
    return _NC_CACHE["nc"]


def _make_in_maps(x, W):
    BFnp = ml_dtypes.bfloat16
    x = np.asarray(x, np.float32)
    W = np.asarray(W, np.float32)
    w_bf = np.ascontiguousarray(W.reshape(KC, 128, O)).astype(BFnp)
    wt = np.ascontiguousarray(W.reshape(C, OG, 128).transpose(1, 2, 0)).astype(BFnp)
    e16 = (np.arange(128)[None, :] % D == np.arange(D)[:, None]).astype(BFnp)
    oo = np.arange(128)
    bn = np.arange(128)
    bm = np.stack(
        [((g * 8 + oo[:, None] // D) == (bn[None, :] % N)) for g in range(OG)]
    ).astype(BFnp)
    dm4 = ((np.arange(O)[None, :] // D) == (bn[:, None] % N)).astype(np.float32)
    # [id128 | on4]: on4[bn, j] = (bn // N == j)
    idon = np.zeros((128, 128 + GB), np.float32)
    idon[:, :128] = np.eye(128)
    idon[bn, 128 + bn // N] = 1.0
    idon = idon.astype(BFnp)
    id128f = np.eye(128, dtype=np.float32)
    cn = np.full((128, 128), 1.0 / N, BFnp)

    in_maps = []
    for core in range(NCORES):
        xs = x[core * BPC:(core + 1) * BPC]              # (8, C, T)
        # (b, c, t) -> [b, 128, (k, t)]
        xbt = np.ascontiguousarray(
            xs.reshape(BPC, KC, 128, T).transpose(0, 2, 1, 3).reshape(
                BPC, 128, KC * T)
        ).astype(BFnp)
        # transposed layout: [g, 128t, (m, b4, c)]
        xt4 = np.zeros((NG, 128, MT * GB * C), BFnp)
        for g in range(NG):
            for b4 in range(GB):
                xtb = xs[g * GB + b4].T                  # (T, C) f32
                blocks = xtb.reshape(MT, 128, C).astype(BFnp)  # (m, tl, c)
                for m in range(MT):
                    xt4[g, :, m * GB * C + b4 * C:(m * GB + b4 + 1) * C] = blocks[m]
        in_maps.append(
            {
                "xb": xbt, "xt": xt4, "wsb": w_bf, "wt": wt, "e16": e16,
                "bm": bm, "dm4": dm4, "idon": idon, "id128f": id128f,
                "cn": cn,
            }
        )
    return in_maps


def run(x, W, trace=False):
    in_maps = _make_in_maps(x, W)
    nc = _get_nc()
    res = run_bass_kernel_spmd(nc, in_maps, core_ids=list(range(NCORES)), trace=trace)
    out = np.concatenate([r["out"] for r in res.results], axis=0)
    return out, res


def kernel(x, W, out_num_capsule=N, out_dim_capsule=D, routings=3, **_):
    out, _res = run(x, W, trace=False)
    return out


# revision 21
# speedup vs baseline: 1.0562x; 1.0063x over previous
"""Capsule routing kernel v3 (Conv1D k=1 -> dynamic routing) for TRN2, 8 cores.

Data-parallel over batch (8 batches/core), 2 groups of 4 batches stacked on
the 128-partition dim as (b,n).  u_hat is never materialized; routing is
factorized through x:
    s[n,d] = sum_c Q[n,c] W[c,nD+d],  Q = c @ x^T        (PE)
    b[n,t] += sum_c P[c,n] x[c,t],    P = W . vmask      (PE)

v3 changes vs v2 (trace-driven):
  * All transposes via PE identity-matmul (lhsT=chunk, rhs=id128) instead of
    serial DMA XBAR transposes (48.6us -> ~4us).  The softmax partition-sums
    ride the same matmul: rhs = [id128 | on4] gives etT and Z^T in one pass.
  * 1/Z via reciprocal_approx_fast on t-major [128,4] chunks (27us -> ~1us).
  * Softmax normalize fused into the transpose psum->sbuf copy as a
    stride-0-broadcast tensor_mul.
  * Iteration 0 (uniform c) via the same Q matmul path with a constant-1/N
    lhsT (kills 18us of DVE reduce_sum, warms the PE during the x DMA).
  * Q matmul col-tiled: 4 concurrent 32-wide strips (tile_position), rhs
    FD=256 per batch; psum comes out already in q_sb layout (no extraction).
  * x loaded as 8+2 big DMAs split across sync and scalar HWDGE queues.
"""

import contextlib

import numpy as np
import ml_dtypes

import concourse.bass as bass
import concourse.tile as tile
from concourse import bacc, mybir
from concourse.bass_utils import run_bass_kernel_spmd

F32 = mybir.dt.float32
BF16 = mybir.dt.bfloat16
AF = mybir.ActivationFunctionType
AX = mybir.AxisListType
ALU = mybir.AluOpType

B, C, T = 64, 256, 1024
N, D = 32, 16
O = N * D            # 512
NCORES = 8
BPC = B // NCORES    # 8 batches per core
NG = 2               # groups per core
GB = 4               # batches per group (stacked as (b,n) on 128 partitions)
KC = C // 128        # 2 contraction chunks
MT = T // 128        # 8 t-chunks
OG = O // 128        # 4 o-chunks
EPS = 1e-7


def _build_bass():
    nc = bacc.Bacc(
        "TRN2",
        target_bir_lowering=False,
        debug=False,
        enable_asserts=False,
        num_devices=NCORES,
    )
    # x in (c,t) layout: per batch one [128, KC*T] tile (cols = (k,t))
    xb_d = nc.dram_tensor("xb", [BPC, 128, KC * T], BF16, kind="ExternalInput").ap()
    # x transposed: per group one [128, MT*GB*C] tile (cols = (m, b4, c))
    xt_d = nc.dram_tensor("xt", [NG, 128, MT * GB * C], BF16, kind="ExternalInput").ap()
    w_d = nc.dram_tensor("wsb", [KC, 128, O], BF16, kind="ExternalInput").ap()
    wt_d = nc.dram_tensor("wt", [OG, 128, C], BF16, kind="ExternalInput").ap()
    e16_d = nc.dram_tensor("e16", [D, 128], BF16, kind="ExternalInput").ap()
    bm_d = nc.dram_tensor("bm", [OG, 128, 128], BF16, kind="ExternalInput").ap()
    dm4_d = nc.dram_tensor("dm4", [128, O], F32, kind="ExternalInput").ap()
    # identity+ones combined rhs for transpose matmuls: [128, 132]
    idon_d = nc.dram_tensor("idon", [128, 128 + GB], BF16, kind="ExternalInput").ap()
    id128f_d = nc.dram_tensor("id128f", [128, 128], F32, kind="ExternalInput").ap()
    cn_d = nc.dram_tensor("cn", [128, 128], BF16, kind="ExternalInput").ap()
    out_d = nc.dram_tensor("out", [BPC, N, D], F32, kind="ExternalOutput").ap()

    with tile.TileContext(nc) as tc:
        _kernel_body(tc, out_d, xb_d, xt_d, w_d, wt_d, e16_d, bm_d, dm4_d,
                     idon_d, id128f_d, cn_d)
    nc.compile()
    return nc


def _kernel_body(tc, out_d, xb_d, xt_d, w_d, wt_d, e16_d, bm_d, dm4_d,
                 idon_d, id128f_d, cn_d):
    nc = tc.nc
    ctx = contextlib.ExitStack()
    with ctx:
        const = ctx.enter_context(tc.tile_pool(name="const", bufs=1))
        xbp = ctx.enter_context(tc.tile_pool(name="xbp", bufs=BPC))
        xtp = ctx.enter_context(tc.tile_pool(name="xtp", bufs=2 * NG))
        lgp = ctx.enter_context(tc.tile_pool(name="lgp", bufs=NG))
        etp = ctx.enter_context(tc.tile_pool(name="etp", bufs=2))
        ctsp = ctx.enter_context(tc.tile_pool(name="ctsp", bufs=2))
        rzp = ctx.enter_context(tc.tile_pool(name="rzp", bufs=2))
        qsp = ctx.enter_context(tc.tile_pool(name="qsp", bufs=2))
        qtp = ctx.enter_context(tc.tile_pool(name="qtp", bufs=4))
        pbp = ctx.enter_context(tc.tile_pool(name="pbp", bufs=4))
        vtp = ctx.enter_context(tc.tile_pool(name="vtp", bufs=2))
        vbp = ctx.enter_context(tc.tile_pool(name="vbp", bufs=2))
        vmp = ctx.enter_context(tc.tile_pool(name="vmp", bufs=8))
        smp = ctx.enter_context(tc.tile_pool(name="smp", bufs=2))
        vp = ctx.enter_context(tc.tile_pool(name="vp", bufs=4))
        tinyp = ctx.enter_context(tc.tile_pool(name="tinyp", bufs=8))
        # PSUM pools
        ptp = ctx.enter_context(tc.tile_pool(name="ptp", bufs=2, space="PSUM"))
        pqt = ctx.enter_context(tc.tile_pool(name="pqt", bufs=1, space="PSUM"))
        pqp = ctx.enter_context(tc.tile_pool(name="pqp", bufs=2, space="PSUM"))
        pband = ctx.enter_context(tc.tile_pool(name="pband", bufs=3, space="PSUM"))

        # --- constants (sync queue) ---
        w_sb = [const.tile([128, O], BF16, name=f"w{k}", tag=f"w{k}") for k in range(KC)]
        for k in range(KC):
            nc.sync.dma_start(w_sb[k][:], w_d[k])
        wt_sb = [const.tile([128, C], BF16, name=f"wt{g}", tag=f"wt{g}") for g in range(OG)]
        for g in range(OG):
            nc.sync.dma_start(wt_sb[g][:], wt_d[g])
        e16 = const.tile([D, 128], BF16, name="e16", tag="e16")
        nc.sync.dma_start(e16[:], e16_d[:])
        bm = [const.tile([128, 128], BF16, name=f"bm{g}", tag=f"bm{g}") for g in range(OG)]
        for g in range(OG):
            nc.sync.dma_start(bm[g][:], bm_d[g])
        dm4 = const.tile([128, O], F32, name="dm4", tag="dm4")
        nc.sync.dma_start(dm4[:], dm4_d[:])
        idon = const.tile([128, 128 + GB], BF16, name="idon", tag="idon")
        nc.sync.dma_start(idon[:], idon_d[:])
        id128f = const.tile([128, 128], F32, name="id128f", tag="id128f")
        nc.sync.dma_start(id128f[:], id128f_d[:])
        cn = const.tile([128, 128], BF16, name="cn", tag="cn")
        nc.sync.dma_start(cn[:], cn_d[:])

        # --- x loads: xt on scalar queue (needed first, split in half for
        # earlier iter-0 start), xb on sync queue ---
        HM = MT // 2
        xth = {}
        for g in range(NG):
            for h in range(2):
                t = xtp.tile([128, HM * GB * C], BF16, name="xt", tag="xt")
                nc.scalar.dma_start(
                    t[:], xt_d[g][:, h * HM * GB * C:(h + 1) * HM * GB * C])
                xth[g, h] = t
        xb = {}
        for b in range(BPC):
            t = xbp.tile([128, KC * T], BF16, name="xb", tag="xb")
            nc.sync.dma_start(t[:], xb_d[b])
            xb[b] = t

        logits = {g: lgp.tile([128, T], F32, name="lg", tag="lg") for g in range(NG)}
        ets = {}

        # scratch for activation-table preloading (keeps the ~1.3us
        # ACT_TABLE_LOAD for Exp<->Sqrt switches off the critical path)
        scr = const.tile([128, 1], F32, name="scr", tag="scr")
        scr2 = const.tile([128, 1], F32, name="scr2", tag="scr2")
        nc.gpsimd.memset(scr[:], 1.0)

        def preload(func, dep=None):
            nc.scalar.activation(scr2[:], scr[:] if dep is None else dep, func)

        def softmax_transpose(g):
            """logits [bn, T] -> cts [128t, (m, bn)] bf16, normalized.

            Per 128-t chunk: one PE matmul with lhsT = exp-chunk and
            rhs = [id128 | on4] yields etT (cols 0:128) and Z^T (cols 128:132)
            in psum; 1/Z via fast reciprocal; normalize fused into the
            psum->sbuf copy as a broadcast multiply.
            """
            lg = logits[g]
            et = etp.tile([128, T], BF16, name="et", tag="et")
            ets[g] = et
            for jh in range(2):
                nc.scalar.activation(
                    et[:, jh * 512:(jh + 1) * 512],
                    lg[:, jh * 512:(jh + 1) * 512], AF.Exp)
            cts = ctsp.tile([128, MT * 128], BF16, name="cts", tag="cts")
            rzt = rzp.tile([128, MT * GB], F32, name="rzt", tag="rzt")
            for m in range(MT):
                tp = ptp.tile([128, 512], F32, name="tp", tag="tp")
                nc.tensor.matmul(
                    tp[:, 0:128 + GB], et[:, m * 128:(m + 1) * 128], idon[:],
                    start=True, stop=True,
                )
                nc.vector.reciprocal(
                    rzt[:, m * GB:(m + 1) * GB], tp[:, 128:128 + GB],
                )
                # cts[:, m-chunk] = etT * (1/Z) broadcast over the 32 n-cols
                nc.vector.tensor_tensor(
                    cts[:, m * 128:(m + 1) * 128].rearrange(
                        "p (b n) -> p b n", n=N),
                    tp[:, 0:128].rearrange("p (b n) -> p b n", n=N),
                    rzt[:, m * GB:(m + 1) * GB].unsqueeze(2).broadcast_to(
                        [128, GB, N]),
                    ALU.mult,
                )
            return cts

        def q_phase(g, cts_ap, cstride):
            """Q[bn, c] col-tiled: strip b4 <- cts chunk-strip ^T @ xt4-slice.

            cts_ap: SBUF AP base; strip (m, b4) slice is
            cts_ap[:, m*cstride + 32*b4 : m*cstride + 32*b4 + 32]
            (cstride=0 for the constant iteration-0 lhsT).
            """
            # Two psum banks: strips {0,1} in qA, {2,3} in qB.  One open
            # accumulation group per bank; pairs (0,2) then (1,3) issue
            # back-to-back at different PE col-groups -> concurrent.
            qA = pqp.tile([128, 512], F32, name="qA", tag="q_ps")
            qB = pqp.tile([128, 512], F32, name="qB", tag="q_ps")
            for phase in range(2):
                for m in range(MT):
                    for b4 in (phase, phase + 2):
                        q_ps = qA if b4 < 2 else qB
                        off = m * cstride + 32 * b4
                        xt_t = xth[g, m // HM]
                        mo = (m % HM) * GB * C + b4 * C
                        nc.tensor.matmul(
                            q_ps[32 * b4:32 * (b4 + 1), 0:C],
                            cts_ap[:, off:off + 32],
                            xt_t[:, mo:mo + C],
                            start=(m == 0), stop=(m == MT - 1),
                            tile_position=(0, 32 * b4),
                        )
            q_sb = qsp.tile([128, C], BF16, name="q_sb", tag="q_sb")
            nc.scalar.copy(q_sb[0:64, :], qA[0:64, 0:C])
            nc.scalar.copy(q_sb[64:128, :], qB[64:128, 0:C])
            # transpose Q via identity matmul, then s = Qt^T @ W
            s_ps = pband.tile([128, O], F32, name="s_ps", tag="band")
            qts = []
            for h in range(KC):
                qt_ps = pqt.tile([128, 512], F32, name="qt_ps", tag="pqt")
                nc.tensor.matmul(
                    qt_ps[:, 0:128], q_sb[:, h * 128:(h + 1) * 128], idon[:, 0:128],
                    start=True, stop=True,
                )
                qt = qtp.tile([128, 128], BF16, name="qt", tag="qt")
                nc.scalar.copy(qt[:], qt_ps[:, 0:128])
                qts.append(qt)
            for h in range(KC):
                nc.tensor.matmul(
                    s_ps[:], qts[h][:], w_sb[h][:],
                    start=(h == 0), stop=(h == KC - 1),
                )
            return s_ps

        def extract_squash(s_ps):
            """psum s_full (128(b,n), O) -> v (128, D) f32 via mask+strided reduce."""
            sm = smp.tile([128, O], F32, name="sm", tag="sm")
            nc.vector.tensor_mul(sm[:], s_ps[:], dm4[:])
            s_t = tinyp.tile([128, D], F32, name="s_t", tag="s_t")
            nc.vector.reduce_sum(
                s_t[:], sm[:].rearrange("p (n d) -> p d n", d=D), axis=AX.X
            )
            sq = tinyp.tile([128, D], F32, name="sq", tag="sq")
            nc.vector.tensor_mul(sq[:], s_t[:], s_t[:])
            s2 = tinyp.tile([128, 1], F32, name="s2", tag="s2")
            nc.vector.reduce_sum(s2[:], sq[:], axis=AX.X)
            s2e = tinyp.tile([128, 1], F32, name="s2e", tag="s2e")
            nc.vector.tensor_scalar_add(s2e[:], s2[:], EPS)
            rt = tinyp.tile([128, 1], F32, name="rt", tag="rt")
            nc.scalar.sqrt(rt[:], s2e[:])
            d1 = tinyp.tile([128, 1], F32, name="d1", tag="d1")
            nc.vector.tensor_scalar_add(d1[:], s2e[:], 1.0)
            r1 = tinyp.tile([128, 1], F32, name="r1", tag="r1")
            nc.vector.reciprocal(r1[:], d1[:])
            sc = tinyp.tile([128, 1], F32, name="sc", tag="sc")
            nc.vector.tensor_mul(sc[:], rt[:], r1[:])
            v = vp.tile([128, D], F32, name="v", tag="v")
            nc.vector.tensor_scalar_mul(v[:], s_t[:], sc[:])
            return v

        def update(g, v, first):
            """logits ((b,n), t) += x^T (W . vmask) for the 4 stacked batches."""
            vt_ps = pqt.tile([128, 512], F32, name="vt_ps", tag="pqt")
            nc.tensor.transpose(vt_ps[0:D, 0:128], v[:], id128f[:])
            vt_bf = vtp.tile([D, 128], BF16, name="vt_bf", tag="vt_bf")
            nc.vector.tensor_copy(vt_bf[:], vt_ps[0:D, 0:128])
            vbc_ps = pqt.tile([128, 512], F32, name="vbc", tag="pqt")
            nc.tensor.matmul(vbc_ps[:, 0:128], e16[:], vt_bf[:], start=True, stop=True)
            vbc_sb = vbp.tile([128, 128], BF16, name="vbc_sb", tag="vbc_sb")
            nc.scalar.copy(vbc_sb[:], vbc_ps[:, 0:128])
            vms = []
            for g4 in range(OG):
                vm = vmp.tile([128, 128], BF16, name="vm", tag="vm")
                nc.vector.tensor_mul(vm[:], vbc_sb[:], bm[g4][:])
                vms.append(vm)
            p_sb = []
            for h in range(KC):
                p_ps = pqt.tile([128, 512], F32, name="p_ps", tag="pqt")
                for g4 in range(OG):
                    nc.tensor.matmul(
                        p_ps[:, 0:128], wt_sb[g4][:, h * 128:(h + 1) * 128], vms[g4][:],
                        start=(g4 == 0), stop=(g4 == OG - 1),
                    )
                pb = pbp.tile([128, 128], BF16, name="pb", tag="pb")
                nc.scalar.copy(pb[:], p_ps[:, 0:128])
                p_sb.append(pb)
            lg = logits[g]
            # two banks (t-halves); strip pairing offset by 1 so concurrent
            # MMs land on different PE col-groups
            a_ps = [
                pband.tile([128, 512], F32, name=f"a_ps{j}", tag="band")
                for j in range(2)
            ]
            for step in range(GB):
                for k in range(KC):
                    for j in range(2):
                        b4 = (step + j) % GB
                        b = g * GB + b4
                        nc.tensor.matmul(
                            a_ps[j][32 * b4:32 * (b4 + 1), :],
                            p_sb[k][:, 32 * b4:32 * (b4 + 1)],
                            xb[b][:, k * T + j * 512:k * T + j * 512 + 512],
                            start=(k == 0), stop=(k == KC - 1),
                            tile_position=(0, 32 * b4),
                        )
            for j in range(2):
                if first:
                    nc.scalar.copy(lg[:, j * 512:(j + 1) * 512], a_ps[j][:])
                else:
                    nc.vector.tensor_add(
                        lg[:, j * 512:(j + 1) * 512],
                        lg[:, j * 512:(j + 1) * 512], a_ps[j][:],
                    )

        # --- iteration 0 (uniform c = 1/N via constant lhsT) ---
        # groups staggered one phase apart so g1's PE phases fill g0's
        # serial (DVE/ACT) chains
        preload(AF.Sqrt)
        sp0 = q_phase(0, cn[:], 0)
        sp1 = q_phase(1, cn[:], 0)
        vs = {0: extract_squash(sp0)}
        update(0, vs[0], first=True)
        vs[1] = extract_squash(sp1)
        preload(AF.Exp, dep=vs[1][:, 0:1])
        update(1, vs[1], first=True)

        # --- iterations 1, 2 ---
        for it in (1, 2):
            cts0 = softmax_transpose(0)
            sp0 = q_phase(0, cts0[:], 128)
            cts1 = softmax_transpose(1)
            preload(AF.Sqrt, dep=ets[1][:, 0:1])
            vs[0] = extract_squash(sp0)
            sp1 = q_phase(1, cts1[:], 128)
            if it == 1:
                update(0, vs[0], first=False)
                vs[1] = extract_squash(sp1)
                preload(AF.Exp, dep=vs[1][:, 0:1])
                update(1, vs[1], first=False)
            else:
                nc.sync.dma_start(out_d[0:GB], vs[0][:])
                vs[1] = extract_squash(sp1)
                nc.sync.dma_start(out_d[GB:2 * GB], vs[1][:])


_NC_CACHE = {}


def _get_nc():
    if "nc" not in _NC_CACHE:
        _NC_CACHE["nc"] = _build_bass()
    return _NC_CACHE["nc"]


def _make_in_maps(x, W):
    BFnp = ml_dtypes.bfloat16
    x = np.asarray(x, np.float32)
    W = np.asarray(W, np.float32)
    w_bf = np.ascontiguousarray(W.reshape(KC, 128, O)).astype(BFnp)
    wt = np.ascontiguousarray(W.reshape(C, OG, 128).transpose(1, 2, 0)).astype(BFnp)
    e16 = (np.arange(128)[None, :] % D == np.arange(D)[:, None]).astype(BFnp)
    oo = np.arange(128)
    bn = np.arange(128)
    bm = np.stack(
        [((g * 8 + oo[:, None] // D) == (bn[None, :] % N)) for g in range(OG)]
    ).astype(BFnp)
    dm4 = ((np.arange(O)[None, :] // D) == (bn[:, None] % N)).astype(np.float32)
    # [id128 | on4]: on4[bn, j] = (bn // N == j)
    idon = np.zeros((128, 128 + GB), np.float32)
    idon[:, :128] = np.eye(128)
    idon[bn, 128 + bn // N] = 1.0
    idon = idon.astype(BFnp)
    id128f = np.eye(128, dtype=np.float32)
    cn = np.full((128, 128), 1.0 / N, BFnp)

    in_maps = []
    for core in range(NCORES):
        xs = x[core * BPC:(core + 1) * BPC]              # (8, C, T)
        # (b, c, t) -> [b, 128, (k, t)]
        xbt = np.ascontiguousarray(
            xs.reshape(BPC, KC, 128, T).transpose(0, 2, 1, 3).reshape(
                BPC, 128, KC * T)
        ).astype(BFnp)
        # transposed layout: [g, 128t, (m, b4, c)]
        xt4 = np.zeros((NG, 128, MT * GB * C), BFnp)
        for g in range(NG):
            for b4 in range(GB):
                xtb = xs[g * GB + b4].T                  # (T, C) f32
                blocks = xtb.reshape(MT, 128, C).astype(BFnp)  # (m, tl, c)
                for m in range(MT):
                    xt4[g, :, m * GB * C + b4 * C:(m * GB + b4 + 1) * C] = blocks[m]
        in_maps.append(
            {
                "xb": xbt, "xt": xt4, "wsb": w_bf, "wt": wt, "e16": e16,
                "bm": bm, "dm4": dm4, "idon": idon, "id128f": id128f,
                "cn": cn,
            }
        )
    return in_maps


def run(x, W, trace=False):
    in_maps = _make_in_maps(x, W)
    nc = _get_nc()
    res = run_bass_kernel_spmd(nc, in_maps, core_ids=list(range(NCORES)), trace=trace)
    out = np.concatenate([r["out"] for r in res.results], axis=0)
    return out, res


def kernel(x, W, out_num_capsule=N, out_dim_capsule=D, routings=3, **_):
    out, _res = run(x, W, trace=False)
    return out


# revision 22
# speedup vs baseline: 1.0688x; 1.0119x over previous
"""Capsule routing kernel v3 (Conv1D k=1 -> dynamic routing) for TRN2, 8 cores.

Data-parallel over batch (8 batches/core), 2 groups of 4 batches stacked on
the 128-partition dim as (b,n).  u_hat is never materialized; routing is
factorized through x:
    s[n,d] = sum_c Q[n,c] W[c,nD+d],  Q = c @ x^T        (PE)
    b[n,t] += sum_c P[c,n] x[c,t],    P = W . vmask      (PE)

v3 changes vs v2 (trace-driven):
  * All transposes via PE identity-matmul (lhsT=chunk, rhs=id128) instead of
    serial DMA XBAR transposes (48.6us -> ~4us).  The softmax partition-sums
    ride the same matmul: rhs = [id128 | on4] gives etT and Z^T in one pass.
  * 1/Z via reciprocal_approx_fast on t-major [128,4] chunks (27us -> ~1us).
  * Softmax normalize fused into the transpose psum->sbuf copy as a
    stride-0-broadcast tensor_mul.
  * Iteration 0 (uniform c) via the same Q matmul path with a constant-1/N
    lhsT (kills 18us of DVE reduce_sum, warms the PE during the x DMA).
  * Q matmul col-tiled: 4 concurrent 32-wide strips (tile_position), rhs
    FD=256 per batch; psum comes out already in q_sb layout (no extraction).
  * x loaded as 8+2 big DMAs split across sync and scalar HWDGE queues.
"""

import contextlib

import numpy as np
import ml_dtypes

import concourse.bass as bass
import concourse.tile as tile
from concourse import bacc, mybir
from concourse.bass_utils import run_bass_kernel_spmd

F32 = mybir.dt.float32
BF16 = mybir.dt.bfloat16
AF = mybir.ActivationFunctionType
AX = mybir.AxisListType
ALU = mybir.AluOpType

B, C, T = 64, 256, 1024
N, D = 32, 16
O = N * D            # 512
NCORES = 8
BPC = B // NCORES    # 8 batches per core
NG = 2               # groups per core
GB = 4               # batches per group (stacked as (b,n) on 128 partitions)
KC = C // 128        # 2 contraction chunks
MT = T // 128        # 8 t-chunks
OG = O // 128        # 4 o-chunks
EPS = 1e-7


def _build_bass():
    nc = bacc.Bacc(
        "TRN2",
        target_bir_lowering=False,
        debug=False,
        enable_asserts=False,
        num_devices=NCORES,
    )
    # x in (c,t) layout: per batch one [128, KC*T] tile (cols = (k,t))
    xb_d = nc.dram_tensor("xb", [BPC, 128, KC * T], BF16, kind="ExternalInput").ap()
    # x transposed: per group one [128, MT*GB*C] tile (cols = (m, b4, c))
    xt_d = nc.dram_tensor("xt", [NG, 128, MT * GB * C], BF16, kind="ExternalInput").ap()
    w_d = nc.dram_tensor("wsb", [KC, 128, O], BF16, kind="ExternalInput").ap()
    wt_d = nc.dram_tensor("wt", [OG, 128, C], BF16, kind="ExternalInput").ap()
    e16_d = nc.dram_tensor("e16", [D, 128], BF16, kind="ExternalInput").ap()
    bm_d = nc.dram_tensor("bm", [OG, 128, 128], BF16, kind="ExternalInput").ap()
    dm4_d = nc.dram_tensor("dm4", [128, O], F32, kind="ExternalInput").ap()
    # identity+ones combined rhs for transpose matmuls: [128, 132]
    idon_d = nc.dram_tensor("idon", [128, 128 + GB], BF16, kind="ExternalInput").ap()
    id128f_d = nc.dram_tensor("id128f", [128, 128], F32, kind="ExternalInput").ap()
    cn_d = nc.dram_tensor("cn", [128, 128], BF16, kind="ExternalInput").ap()
    out_d = nc.dram_tensor("out", [BPC, N, D], F32, kind="ExternalOutput").ap()

    with tile.TileContext(nc) as tc:
        _kernel_body(tc, out_d, xb_d, xt_d, w_d, wt_d, e16_d, bm_d, dm4_d,
                     idon_d, id128f_d, cn_d)
    nc.compile()
    return nc


def _kernel_body(tc, out_d, xb_d, xt_d, w_d, wt_d, e16_d, bm_d, dm4_d,
                 idon_d, id128f_d, cn_d):
    nc = tc.nc
    ctx = contextlib.ExitStack()
    with ctx:
        const = ctx.enter_context(tc.tile_pool(name="const", bufs=1))
        xbp = ctx.enter_context(tc.tile_pool(name="xbp", bufs=BPC))
        xtp = ctx.enter_context(tc.tile_pool(name="xtp", bufs=2 * NG))
        lgp = ctx.enter_context(tc.tile_pool(name="lgp", bufs=NG))
        etp = ctx.enter_context(tc.tile_pool(name="etp", bufs=2))
        ctsp = ctx.enter_context(tc.tile_pool(name="ctsp", bufs=2))
        rzp = ctx.enter_context(tc.tile_pool(name="rzp", bufs=2))
        qsp = ctx.enter_context(tc.tile_pool(name="qsp", bufs=2))
        qtp = ctx.enter_context(tc.tile_pool(name="qtp", bufs=4))
        pbp = ctx.enter_context(tc.tile_pool(name="pbp", bufs=4))
        vtp = ctx.enter_context(tc.tile_pool(name="vtp", bufs=2))
        vbp = ctx.enter_context(tc.tile_pool(name="vbp", bufs=2))
        vmp = ctx.enter_context(tc.tile_pool(name="vmp", bufs=8))
        smp = ctx.enter_context(tc.tile_pool(name="smp", bufs=2))
        vp = ctx.enter_context(tc.tile_pool(name="vp", bufs=4))
        tinyp = ctx.enter_context(tc.tile_pool(name="tinyp", bufs=8))
        # PSUM pools
        ptp = ctx.enter_context(tc.tile_pool(name="ptp", bufs=2, space="PSUM"))
        pqt = ctx.enter_context(tc.tile_pool(name="pqt", bufs=1, space="PSUM"))
        pqp = ctx.enter_context(tc.tile_pool(name="pqp", bufs=2, space="PSUM"))
        pband = ctx.enter_context(tc.tile_pool(name="pband", bufs=3, space="PSUM"))

        # --- loads.  iter-0-critical tensors first (cn, idon, xt g0) so
        # their DMA-completion semaphore targets are small and the first Q
        # matmuls can start as soon as those transfers land. ---
        cn = const.tile([128, 128], BF16, name="cn", tag="cn")
        nc.sync.dma_start(cn[:], cn_d[:])
        idon = const.tile([128, 128 + GB], BF16, name="idon", tag="idon")
        nc.sync.dma_start(idon[:], idon_d[:])
        HM = MT // 2
        xth = {}
        for g in range(NG):
            for h in range(2):
                t = xtp.tile([128, HM * GB * C], BF16, name="xt", tag="xt")
                nc.scalar.dma_start(
                    t[:], xt_d[g][:, h * HM * GB * C:(h + 1) * HM * GB * C])
                xth[g, h] = t
        w_sb = [const.tile([128, O], BF16, name=f"w{k}", tag=f"w{k}") for k in range(KC)]
        for k in range(KC):
            nc.sync.dma_start(w_sb[k][:], w_d[k])
        dm4 = const.tile([128, O], F32, name="dm4", tag="dm4")
        nc.sync.dma_start(dm4[:], dm4_d[:])
        wt_sb = [const.tile([128, C], BF16, name=f"wt{g}", tag=f"wt{g}") for g in range(OG)]
        for g in range(OG):
            nc.sync.dma_start(wt_sb[g][:], wt_d[g])
        e16 = const.tile([D, 128], BF16, name="e16", tag="e16")
        nc.sync.dma_start(e16[:], e16_d[:])
        bm = [const.tile([128, 128], BF16, name=f"bm{g}", tag=f"bm{g}") for g in range(OG)]
        for g in range(OG):
            nc.sync.dma_start(bm[g][:], bm_d[g])
        id128f = const.tile([128, 128], F32, name="id128f", tag="id128f")
        nc.sync.dma_start(id128f[:], id128f_d[:])
        xb = {}
        for b in range(BPC):
            t = xbp.tile([128, KC * T], BF16, name="xb", tag="xb")
            nc.sync.dma_start(t[:], xb_d[b])
            xb[b] = t

        logits = {g: lgp.tile([128, T], F32, name="lg", tag="lg") for g in range(NG)}
        ets = {}

        # scratch for activation-table preloading (keeps the ~1.3us
        # ACT_TABLE_LOAD for Exp<->Sqrt switches off the critical path)
        scr = const.tile([128, 1], F32, name="scr", tag="scr")
        scr2 = const.tile([128, 1], F32, name="scr2", tag="scr2")
        nc.gpsimd.memset(scr[:], 1.0)

        def preload(func, dep=None):
            nc.scalar.activation(scr2[:], scr[:] if dep is None else dep, func)

        def softmax_transpose(g):
            """logits [bn, T] -> cts [128t, (m, bn)] bf16, normalized.

            Per 128-t chunk: one PE matmul with lhsT = exp-chunk and
            rhs = [id128 | on4] yields etT (cols 0:128) and Z^T (cols 128:132)
            in psum; 1/Z via fast reciprocal; normalize fused into the
            psum->sbuf copy as a broadcast multiply.
            """
            lg = logits[g]
            et = etp.tile([128, T], BF16, name="et", tag="et")
            ets[g] = et
            for jh in range(2):
                nc.scalar.activation(
                    et[:, jh * 512:(jh + 1) * 512],
                    lg[:, jh * 512:(jh + 1) * 512], AF.Exp)
            cts = ctsp.tile([128, MT * 128], BF16, name="cts", tag="cts")
            rzt = rzp.tile([128, MT * GB], F32, name="rzt", tag="rzt")
            for m in range(MT):
                tp = ptp.tile([128, 512], F32, name="tp", tag="tp")
                nc.tensor.matmul(
                    tp[:, 0:128 + GB], et[:, m * 128:(m + 1) * 128], idon[:],
                    start=True, stop=True,
                )
                nc.vector.reciprocal(
                    rzt[:, m * GB:(m + 1) * GB], tp[:, 128:128 + GB],
                )
                # cts[:, m-chunk] = etT * (1/Z) broadcast over the 32 n-cols
                nc.vector.tensor_tensor(
                    cts[:, m * 128:(m + 1) * 128].rearrange(
                        "p (b n) -> p b n", n=N),
                    tp[:, 0:128].rearrange("p (b n) -> p b n", n=N),
                    rzt[:, m * GB:(m + 1) * GB].unsqueeze(2).broadcast_to(
                        [128, GB, N]),
                    ALU.mult,
                )
            return cts

        def q_phase(g, cts_ap, cstride):
            """Q[bn, c] col-tiled: strip b4 <- cts chunk-strip ^T @ xt4-slice.

            cts_ap: SBUF AP base; strip (m, b4) slice is
            cts_ap[:, m*cstride + 32*b4 : m*cstride + 32*b4 + 32]
            (cstride=0 for the constant iteration-0 lhsT).
            """
            # Two psum banks: strips {0,1} in qA, {2,3} in qB.  One open
            # accumulation group per bank; pairs (0,2) then (1,3) issue
            # back-to-back at different PE col-groups -> concurrent.
            qA = pqp.tile([128, 512], F32, name="qA", tag="q_ps")
            qB = pqp.tile([128, 512], F32, name="qB", tag="q_ps")
            for phase in range(2):
                for m in range(MT):
                    for b4 in (phase, phase + 2):
                        q_ps = qA if b4 < 2 else qB
                        off = m * cstride + 32 * b4
                        xt_t = xth[g, m // HM]
                        mo = (m % HM) * GB * C + b4 * C
                        nc.tensor.matmul(
                            q_ps[32 * b4:32 * (b4 + 1), 0:C],
                            cts_ap[:, off:off + 32],
                            xt_t[:, mo:mo + C],
                            start=(m == 0), stop=(m == MT - 1),
                            tile_position=(0, 32 * b4),
                        )
            q_sb = qsp.tile([128, C], BF16, name="q_sb", tag="q_sb")
            nc.scalar.copy(q_sb[0:64, :], qA[0:64, 0:C])
            nc.scalar.copy(q_sb[64:128, :], qB[64:128, 0:C])
            # transpose Q via identity matmul, then s = Qt^T @ W
            s_ps = pband.tile([128, O], F32, name="s_ps", tag="band")
            qts = []
            for h in range(KC):
                qt_ps = pqt.tile([128, 512], F32, name="qt_ps", tag="pqt")
                nc.tensor.matmul(
                    qt_ps[:, 0:128], q_sb[:, h * 128:(h + 1) * 128], idon[:, 0:128],
                    start=True, stop=True,
                )
                qt = qtp.tile([128, 128], BF16, name="qt", tag="qt")
                nc.scalar.copy(qt[:], qt_ps[:, 0:128])
                qts.append(qt)
            for h in range(KC):
                nc.tensor.matmul(
                    s_ps[:], qts[h][:], w_sb[h][:],
                    start=(h == 0), stop=(h == KC - 1),
                )
            return s_ps

        def extract_squash(s_ps):
            """psum s_full (128(b,n), O) -> v (128, D) f32 via mask+strided reduce."""
            sm = smp.tile([128, O], F32, name="sm", tag="sm")
            nc.vector.tensor_mul(sm[:], s_ps[:], dm4[:])
            s_t = tinyp.tile([128, D], F32, name="s_t", tag="s_t")
            nc.vector.reduce_sum(
                s_t[:], sm[:].rearrange("p (n d) -> p d n", d=D), axis=AX.X
            )
            sq = tinyp.tile([128, D], F32, name="sq", tag="sq")
            nc.vector.tensor_mul(sq[:], s_t[:], s_t[:])
            s2 = tinyp.tile([128, 1], F32, name="s2", tag="s2")
            nc.vector.reduce_sum(s2[:], sq[:], axis=AX.X)
            s2e = tinyp.tile([128, 1], F32, name="s2e", tag="s2e")
            nc.vector.tensor_scalar_add(s2e[:], s2[:], EPS)
            rt = tinyp.tile([128, 1], F32, name="rt", tag="rt")
            nc.scalar.sqrt(rt[:], s2e[:])
            d1 = tinyp.tile([128, 1], F32, name="d1", tag="d1")
            nc.vector.tensor_scalar_add(d1[:], s2e[:], 1.0)
            r1 = tinyp.tile([128, 1], F32, name="r1", tag="r1")
            nc.vector.reciprocal(r1[:], d1[:])
            sc = tinyp.tile([128, 1], F32, name="sc", tag="sc")
            nc.vector.tensor_mul(sc[:], rt[:], r1[:])
            v = vp.tile([128, D], F32, name="v", tag="v")
            nc.vector.tensor_scalar_mul(v[:], s_t[:], sc[:])
            return v

        def update(g, v, first):
            """logits ((b,n), t) += x^T (W . vmask) for the 4 stacked batches."""
            vt_ps = pqt.tile([128, 512], F32, name="vt_ps", tag="pqt")
            nc.tensor.transpose(vt_ps[0:D, 0:128], v[:], id128f[:])
            vt_bf = vtp.tile([D, 128], BF16, name="vt_bf", tag="vt_bf")
            nc.vector.tensor_copy(vt_bf[:], vt_ps[0:D, 0:128])
            vbc_ps = pqt.tile([128, 512], F32, name="vbc", tag="pqt")
            nc.tensor.matmul(vbc_ps[:, 0:128], e16[:], vt_bf[:], start=True, stop=True)
            vbc_sb = vbp.tile([128, 128], BF16, name="vbc_sb", tag="vbc_sb")
            nc.scalar.copy(vbc_sb[:], vbc_ps[:, 0:128])
            vms = []
            for g4 in range(OG):
                vm = vmp.tile([128, 128], BF16, name="vm", tag="vm")
                nc.vector.tensor_mul(vm[:], vbc_sb[:], bm[g4][:])
                vms.append(vm)
            p_sb = []
            for h in range(KC):
                p_ps = pqt.tile([128, 512], F32, name="p_ps", tag="pqt")
                for g4 in range(OG):
                    nc.tensor.matmul(
                        p_ps[:, 0:128], wt_sb[g4][:, h * 128:(h + 1) * 128], vms[g4][:],
                        start=(g4 == 0), stop=(g4 == OG - 1),
                    )
                pb = pbp.tile([128, 128], BF16, name="pb", tag="pb")
                nc.scalar.copy(pb[:], p_ps[:, 0:128])
                p_sb.append(pb)
            lg = logits[g]
            # two banks (t-halves); strip pairing offset by 1 so concurrent
            # MMs land on different PE col-groups
            a_ps = [
                pband.tile([128, 512], F32, name=f"a_ps{j}", tag="band")
                for j in range(2)
            ]
            for step in range(GB):
                for k in range(KC):
                    for j in range(2):
                        b4 = (step + j) % GB
                        b = g * GB + b4
                        nc.tensor.matmul(
                            a_ps[j][32 * b4:32 * (b4 + 1), :],
                            p_sb[k][:, 32 * b4:32 * (b4 + 1)],
                            xb[b][:, k * T + j * 512:k * T + j * 512 + 512],
                            start=(k == 0), stop=(k == KC - 1),
                            tile_position=(0, 32 * b4),
                        )
            for j in range(2):
                if first:
                    nc.scalar.copy(lg[:, j * 512:(j + 1) * 512], a_ps[j][:])
                else:
                    nc.vector.tensor_add(
                        lg[:, j * 512:(j + 1) * 512],
                        lg[:, j * 512:(j + 1) * 512], a_ps[j][:],
                    )

        # --- iteration 0 (uniform c = 1/N via constant lhsT) ---
        # groups staggered one phase apart so g1's PE phases fill g0's
        # serial (DVE/ACT) chains
        preload(AF.Sqrt)
        sp0 = q_phase(0, cn[:], 0)
        sp1 = q_phase(1, cn[:], 0)
        vs = {0: extract_squash(sp0)}
        update(0, vs[0], first=True)
        vs[1] = extract_squash(sp1)
        preload(AF.Exp, dep=vs[1][:, 0:1])
        update(1, vs[1], first=True)

        # --- iterations 1, 2 ---
        for it in (1, 2):
            cts0 = softmax_transpose(0)
            sp0 = q_phase(0, cts0[:], 128)
            cts1 = softmax_transpose(1)
            preload(AF.Sqrt, dep=ets[1][:, 0:1])
            vs[0] = extract_squash(sp0)
            sp1 = q_phase(1, cts1[:], 128)
            if it == 1:
                update(0, vs[0], first=False)
                vs[1] = extract_squash(sp1)
                preload(AF.Exp, dep=vs[1][:, 0:1])
                update(1, vs[1], first=False)
            else:
                nc.sync.dma_start(out_d[0:GB], vs[0][:])
                vs[1] = extract_squash(sp1)
                nc.sync.dma_start(out_d[GB:2 * GB], vs[1][:])


_NC_CACHE = {}


def _get_nc():
    if "nc" not in _NC_CACHE:
        _NC_CACHE["nc"] = _build_bass()
    return _NC_CACHE["nc"]


def _make_in_maps(x, W):
    BFnp = ml_dtypes.bfloat16
    x = np.asarray(x, np.float32)
    W = np.asarray(W, np.float32)
    w_bf = np.ascontiguousarray(W.reshape(KC, 128, O)).astype(BFnp)
    wt = np.ascontiguousarray(W.reshape(C, OG, 128).transpose(1, 2, 0)).astype(BFnp)
    e16 = (np.arange(128)[None, :] % D == np.arange(D)[:, None]).astype(BFnp)
    oo = np.arange(128)
    bn = np.arange(128)
    bm = np.stack(
        [((g * 8 + oo[:, None] // D) == (bn[None, :] % N)) for g in range(OG)]
    ).astype(BFnp)
    dm4 = ((np.arange(O)[None, :] // D) == (bn[:, None] % N)).astype(np.float32)
    # [id128 | on4]: on4[bn, j] = (bn // N == j)
    idon = np.zeros((128, 128 + GB), np.float32)
    idon[:, :128] = np.eye(128)
    idon[bn, 128 + bn // N] = 1.0
    idon = idon.astype(BFnp)
    id128f = np.eye(128, dtype=np.float32)
    cn = np.full((128, 128), 1.0 / N, BFnp)

    in_maps = []
    for core in range(NCORES):
        xs = x[core * BPC:(core + 1) * BPC]              # (8, C, T)
        # (b, c, t) -> [b, 128, (k, t)]
        xbt = np.ascontiguousarray(
            xs.reshape(BPC, KC, 128, T).transpose(0, 2, 1, 3).reshape(
                BPC, 128, KC * T)
        ).astype(BFnp)
        # transposed layout: [g, 128t, (m, b4, c)]
        xt4 = np.zeros((NG, 128, MT * GB * C), BFnp)
        for g in range(NG):
            for b4 in range(GB):
                xtb = xs[g * GB + b4].T                  # (T, C) f32
                blocks = xtb.reshape(MT, 128, C).astype(BFnp)  # (m, tl, c)
                for m in range(MT):
                    xt4[g, :, m * GB * C + b4 * C:(m * GB + b4 + 1) * C] = blocks[m]
        in_maps.append(
            {
                "xb": xbt, "xt": xt4, "wsb": w_bf, "wt": wt, "e16": e16,
                "bm": bm, "dm4": dm4, "idon": idon, "id128f": id128f,
                "cn": cn,
            }
        )
    return in_maps


def run(x, W, trace=False):
    in_maps = _make_in_maps(x, W)
    nc = _get_nc()
    res = run_bass_kernel_spmd(nc, in_maps, core_ids=list(range(NCORES)), trace=trace)
    out = np.concatenate([r["out"] for r in res.results], axis=0)
    return out, res


def kernel(x, W, out_num_capsule=N, out_dim_capsule=D, routings=3, **_):
    out, _res = run(x, W, trace=False)
    return out


# revision 23
# speedup vs baseline: 1.1088x; 1.0374x over previous
"""Capsule routing kernel v3 (Conv1D k=1 -> dynamic routing) for TRN2, 8 cores.

Data-parallel over batch (8 batches/core), 2 groups of 4 batches stacked on
the 128-partition dim as (b,n).  u_hat is never materialized; routing is
factorized through x:
    s[n,d] = sum_c Q[n,c] W[c,nD+d],  Q = c @ x^T        (PE)
    b[n,t] += sum_c P[c,n] x[c,t],    P = W . vmask      (PE)

v3 changes vs v2 (trace-driven):
  * All transposes via PE identity-matmul (lhsT=chunk, rhs=id128) instead of
    serial DMA XBAR transposes (48.6us -> ~4us).  The softmax partition-sums
    ride the same matmul: rhs = [id128 | on4] gives etT and Z^T in one pass.
  * 1/Z via reciprocal_approx_fast on t-major [128,4] chunks (27us -> ~1us).
  * Softmax normalize fused into the transpose psum->sbuf copy as a
    stride-0-broadcast tensor_mul.
  * Iteration 0 (uniform c) via the same Q matmul path with a constant-1/N
    lhsT (kills 18us of DVE reduce_sum, warms the PE during the x DMA).
  * Q matmul col-tiled: 4 concurrent 32-wide strips (tile_position), rhs
    FD=256 per batch; psum comes out already in q_sb layout (no extraction).
  * x loaded as 8+2 big DMAs split across sync and scalar HWDGE queues.
"""

import contextlib

import numpy as np
import ml_dtypes

import concourse.bass as bass
import concourse.tile as tile
from concourse import bacc, mybir
from concourse.bass_utils import run_bass_kernel_spmd

F32 = mybir.dt.float32
BF16 = mybir.dt.bfloat16
AF = mybir.ActivationFunctionType
AX = mybir.AxisListType
ALU = mybir.AluOpType

B, C, T = 64, 256, 1024
N, D = 32, 16
O = N * D            # 512
NCORES = 8
BPC = B // NCORES    # 8 batches per core
NG = 2               # groups per core
GB = 4               # batches per group (stacked as (b,n) on 128 partitions)
KC = C // 128        # 2 contraction chunks
MT = T // 128        # 8 t-chunks
OG = O // 128        # 4 o-chunks
EPS = 1e-7


def _build_bass():
    nc = bacc.Bacc(
        "TRN2",
        target_bir_lowering=False,
        debug=False,
        enable_asserts=False,
        num_devices=NCORES,
    )
    # x in (c,t) layout: per batch one [128, KC*T] tile (cols = (k,t))
    xb_d = nc.dram_tensor("xb", [BPC, 128, KC * T], BF16, kind="ExternalInput").ap()
    # x transposed: per group one [128, MT*GB*C] tile (cols = (m, b4, c))
    xt_d = nc.dram_tensor("xt", [NG, 128, MT * GB * C], BF16, kind="ExternalInput").ap()
    w_d = nc.dram_tensor("wsb", [KC, 128, O], BF16, kind="ExternalInput").ap()
    wt_d = nc.dram_tensor("wt", [OG, 128, C], BF16, kind="ExternalInput").ap()
    e16_d = nc.dram_tensor("e16", [D, 128], BF16, kind="ExternalInput").ap()
    bm_d = nc.dram_tensor("bm", [OG, 128, 128], BF16, kind="ExternalInput").ap()
    dm4_d = nc.dram_tensor("dm4", [128, O], F32, kind="ExternalInput").ap()
    # identity+ones combined rhs for transpose matmuls: [128, 132]
    idon_d = nc.dram_tensor("idon", [128, 128 + GB], BF16, kind="ExternalInput").ap()
    id128f_d = nc.dram_tensor("id128f", [128, 128], F32, kind="ExternalInput").ap()
    cn_d = nc.dram_tensor("cn", [128, 128], BF16, kind="ExternalInput").ap()
    out_d = nc.dram_tensor("out", [BPC, N, D], F32, kind="ExternalOutput").ap()

    with tile.TileContext(nc) as tc:
        _kernel_body(tc, out_d, xb_d, xt_d, w_d, wt_d, e16_d, bm_d, dm4_d,
                     idon_d, id128f_d, cn_d)
    nc.compile()
    return nc


def _kernel_body(tc, out_d, xb_d, xt_d, w_d, wt_d, e16_d, bm_d, dm4_d,
                 idon_d, id128f_d, cn_d):
    nc = tc.nc
    ctx = contextlib.ExitStack()
    with ctx:
        const = ctx.enter_context(tc.tile_pool(name="const", bufs=1))
        xbp = ctx.enter_context(tc.tile_pool(name="xbp", bufs=BPC))
        xtp = ctx.enter_context(tc.tile_pool(name="xtp", bufs=2 * NG))
        lgp = ctx.enter_context(tc.tile_pool(name="lgp", bufs=NG))
        etp = ctx.enter_context(tc.tile_pool(name="etp", bufs=2))
        ctsp = ctx.enter_context(tc.tile_pool(name="ctsp", bufs=2))
        rzp = ctx.enter_context(tc.tile_pool(name="rzp", bufs=2))
        qsp = ctx.enter_context(tc.tile_pool(name="qsp", bufs=2))
        qtp = ctx.enter_context(tc.tile_pool(name="qtp", bufs=4))
        pbp = ctx.enter_context(tc.tile_pool(name="pbp", bufs=4))
        vtp = ctx.enter_context(tc.tile_pool(name="vtp", bufs=2))
        vbp = ctx.enter_context(tc.tile_pool(name="vbp", bufs=2))
        vmp = ctx.enter_context(tc.tile_pool(name="vmp", bufs=8))
        smp = ctx.enter_context(tc.tile_pool(name="smp", bufs=2))
        vp = ctx.enter_context(tc.tile_pool(name="vp", bufs=4))
        tinyp = ctx.enter_context(tc.tile_pool(name="tinyp", bufs=8))
        # PSUM pools
        ptp = ctx.enter_context(tc.tile_pool(name="ptp", bufs=2, space="PSUM"))
        pqt = ctx.enter_context(tc.tile_pool(name="pqt", bufs=1, space="PSUM"))
        pqp = ctx.enter_context(tc.tile_pool(name="pqp", bufs=2, space="PSUM"))
        pband = ctx.enter_context(tc.tile_pool(name="pband", bufs=3, space="PSUM"))

        # --- loads.  iter-0-critical tensors first (cn, idon, xt g0) so
        # their DMA-completion semaphore targets are small and the first Q
        # matmuls can start as soon as those transfers land. ---
        cn = const.tile([128, 128], BF16, name="cn", tag="cn")
        nc.sync.dma_start(cn[:], cn_d[:])
        idon = const.tile([128, 128 + GB], BF16, name="idon", tag="idon")
        nc.sync.dma_start(idon[:], idon_d[:])
        HM = MT // 2
        xth = {}
        for g in range(NG):
            for h in range(2):
                t = xtp.tile([128, HM * GB * C], BF16, name="xt", tag="xt")
                nc.scalar.dma_start(
                    t[:], xt_d[g][:, h * HM * GB * C:(h + 1) * HM * GB * C])
                xth[g, h] = t
        w_sb = [const.tile([128, O], BF16, name=f"w{k}", tag=f"w{k}") for k in range(KC)]
        for k in range(KC):
            nc.sync.dma_start(w_sb[k][:], w_d[k])
        dm4 = const.tile([128, O], F32, name="dm4", tag="dm4")
        nc.sync.dma_start(dm4[:], dm4_d[:])
        wt_sb = [const.tile([128, C], BF16, name=f"wt{g}", tag=f"wt{g}") for g in range(OG)]
        for g in range(OG):
            nc.sync.dma_start(wt_sb[g][:], wt_d[g])
        e16 = const.tile([D, 128], BF16, name="e16", tag="e16")
        nc.sync.dma_start(e16[:], e16_d[:])
        bm = [const.tile([128, 128], BF16, name=f"bm{g}", tag=f"bm{g}") for g in range(OG)]
        for g in range(OG):
            nc.sync.dma_start(bm[g][:], bm_d[g])
        id128f = const.tile([128, 128], F32, name="id128f", tag="id128f")
        nc.sync.dma_start(id128f[:], id128f_d[:])
        xb = {}
        for b in range(BPC):
            t = xbp.tile([128, KC * T], BF16, name="xb", tag="xb")
            nc.sync.dma_start(t[:], xb_d[b])
            xb[b] = t

        logits = {g: lgp.tile([128, T], F32, name="lg", tag="lg") for g in range(NG)}
        ets = {}

        # scratch for activation-table preloading (keeps the ~1.3us
        # ACT_TABLE_LOAD for Exp<->Sqrt switches off the critical path)
        scr = const.tile([128, 1], F32, name="scr", tag="scr")
        scr2 = const.tile([128, 1], F32, name="scr2", tag="scr2")
        nc.gpsimd.memset(scr[:], 1.0)

        def preload(func, dep=None):
            nc.scalar.activation(scr2[:], scr[:] if dep is None else dep, func)

        def softmax_transpose(g):
            """logits [bn, T] -> cts [128t, (m, bn)] bf16, normalized.

            Per 128-t chunk: one PE matmul with lhsT = exp-chunk and
            rhs = [id128 | on4] yields etT (cols 0:128) and Z^T (cols 128:132)
            in psum; 1/Z via fast reciprocal; normalize fused into the
            psum->sbuf copy as a broadcast multiply.
            """
            lg = logits[g]
            et = etp.tile([128, T], BF16, name="et", tag="et")
            ets[g] = et
            for jh in range(2):
                nc.scalar.activation(
                    et[:, jh * 512:(jh + 1) * 512],
                    lg[:, jh * 512:(jh + 1) * 512], AF.Exp)
            cts = ctsp.tile([128, MT * 128], BF16, name="cts", tag="cts")
            rzt = rzp.tile([128, MT * GB], F32, name="rzt", tag="rzt")
            for m in range(MT):
                tp = ptp.tile([128, 512], F32, name="tp", tag="tp")
                nc.tensor.matmul(
                    tp[:, 0:128 + GB], et[:, m * 128:(m + 1) * 128], idon[:],
                    start=True, stop=True,
                )
                nc.vector.reciprocal(
                    rzt[:, m * GB:(m + 1) * GB], tp[:, 128:128 + GB],
                )
                # cts[:, m-chunk] = etT * (1/Z) broadcast over the 32 n-cols
                nc.vector.tensor_tensor(
                    cts[:, m * 128:(m + 1) * 128].rearrange(
                        "p (b n) -> p b n", n=N),
                    tp[:, 0:128].rearrange("p (b n) -> p b n", n=N),
                    rzt[:, m * GB:(m + 1) * GB].unsqueeze(2).broadcast_to(
                        [128, GB, N]),
                    ALU.mult,
                )
            return cts

        def q_phase(g, cts_ap, cstride):
            """Q[bn, c] col-tiled: strip b4 <- cts chunk-strip ^T @ xt4-slice.

            cts_ap: SBUF AP base; strip (m, b4) slice is
            cts_ap[:, m*cstride + 32*b4 : m*cstride + 32*b4 + 32]
            (cstride=0 for the constant iteration-0 lhsT).
            """
            # Two psum banks: strips {0,1} in qA, {2,3} in qB.  One open
            # accumulation group per bank; pairs (0,2) then (1,3) issue
            # back-to-back at different PE col-groups -> concurrent.
            qA = pqp.tile([128, 512], F32, name="qA", tag="q_ps")
            qB = pqp.tile([128, 512], F32, name="qB", tag="q_ps")
            for phase in range(2):
                for m in range(MT):
                    for b4 in (phase, phase + 2):
                        q_ps = qA if b4 < 2 else qB
                        off = m * cstride + 32 * b4
                        xt_t = xth[g, m // HM]
                        mo = (m % HM) * GB * C + b4 * C
                        nc.tensor.matmul(
                            q_ps[32 * b4:32 * (b4 + 1), 0:C],
                            cts_ap[:, off:off + 32],
                            xt_t[:, mo:mo + C],
                            start=(m == 0), stop=(m == MT - 1),
                            tile_position=(0, 32 * b4),
                        )
            q_sb = qsp.tile([128, C], BF16, name="q_sb", tag="q_sb")
            nc.scalar.copy(q_sb[0:64, :], qA[0:64, 0:C])
            nc.scalar.copy(q_sb[64:128, :], qB[64:128, 0:C])
            # transpose Q via identity matmul, then s = Qt^T @ W
            s_ps = pband.tile([128, O], F32, name="s_ps", tag="band")
            qts = []
            for h in range(KC):
                qt_ps = pqp.tile([128, 512], F32, name="qt_ps", tag="q_ps")
                nc.tensor.matmul(
                    qt_ps[:, 0:128], q_sb[:, h * 128:(h + 1) * 128], idon[:, 0:128],
                    start=True, stop=True,
                )
                qt = qtp.tile([128, 128], BF16, name="qt", tag="qt")
                nc.scalar.copy(qt[:], qt_ps[:, 0:128])
                qts.append(qt)
            for h in range(KC):
                nc.tensor.matmul(
                    s_ps[:], qts[h][:], w_sb[h][:],
                    start=(h == 0), stop=(h == KC - 1),
                )
            return s_ps

        def extract_squash(s_ps):
            """psum s_full (128(b,n), O) -> v (128, D) f32 via mask+strided reduce."""
            sm = smp.tile([128, O], F32, name="sm", tag="sm")
            nc.vector.tensor_mul(sm[:], s_ps[:], dm4[:])
            s_t = tinyp.tile([128, D], F32, name="s_t", tag="s_t")
            nc.vector.reduce_sum(
                s_t[:], sm[:].rearrange("p (n d) -> p d n", d=D), axis=AX.X
            )
            sq = tinyp.tile([128, D], F32, name="sq", tag="sq")
            nc.vector.tensor_mul(sq[:], s_t[:], s_t[:])
            s2 = tinyp.tile([128, 1], F32, name="s2", tag="s2")
            nc.vector.reduce_sum(s2[:], sq[:], axis=AX.X)
            s2e = tinyp.tile([128, 1], F32, name="s2e", tag="s2e")
            nc.vector.tensor_scalar_add(s2e[:], s2[:], EPS)
            rt = tinyp.tile([128, 1], F32, name="rt", tag="rt")
            nc.scalar.sqrt(rt[:], s2e[:])
            d1 = tinyp.tile([128, 1], F32, name="d1", tag="d1")
            nc.vector.tensor_scalar_add(d1[:], s2e[:], 1.0)
            r1 = tinyp.tile([128, 1], F32, name="r1", tag="r1")
            nc.vector.reciprocal(r1[:], d1[:])
            sc = tinyp.tile([128, 1], F32, name="sc", tag="sc")
            nc.vector.tensor_mul(sc[:], rt[:], r1[:])
            v = vp.tile([128, D], F32, name="v", tag="v")
            nc.vector.tensor_scalar_mul(v[:], s_t[:], sc[:])
            return v

        def update(g, v, first):
            """logits ((b,n), t) += x^T (W . vmask) for the 4 stacked batches."""
            vt_ps = pqt.tile([128, 512], F32, name="vt_ps", tag="pqt")
            nc.tensor.transpose(vt_ps[0:D, 0:128], v[:], id128f[:])
            vt_bf = vtp.tile([D, 128], BF16, name="vt_bf", tag="vt_bf")
            nc.vector.tensor_copy(vt_bf[:], vt_ps[0:D, 0:128])
            vbc_ps = pqt.tile([128, 512], F32, name="vbc", tag="pqt")
            nc.tensor.matmul(vbc_ps[:, 0:128], e16[:], vt_bf[:], start=True, stop=True)
            vbc_sb = vbp.tile([128, 128], BF16, name="vbc_sb", tag="vbc_sb")
            nc.scalar.copy(vbc_sb[:], vbc_ps[:, 0:128])
            vms = []
            for g4 in range(OG):
                vm = vmp.tile([128, 128], BF16, name="vm", tag="vm")
                nc.vector.tensor_mul(vm[:], vbc_sb[:], bm[g4][:])
                vms.append(vm)
            p_sb = []
            for h in range(KC):
                p_ps = ptp.tile([128, 512], F32, name="p_ps", tag="tp")
                for g4 in range(OG):
                    nc.tensor.matmul(
                        p_ps[:, 0:128], wt_sb[g4][:, h * 128:(h + 1) * 128], vms[g4][:],
                        start=(g4 == 0), stop=(g4 == OG - 1),
                    )
                pb = pbp.tile([128, 128], BF16, name="pb", tag="pb")
                nc.scalar.copy(pb[:], p_ps[:, 0:128])
                p_sb.append(pb)
            lg = logits[g]
            # two banks (t-halves); strip pairing offset by 1 so concurrent
            # MMs land on different PE col-groups
            a_ps = [
                pband.tile([128, 512], F32, name=f"a_ps{j}", tag="band")
                for j in range(2)
            ]
            for step in range(GB):
                for k in range(KC):
                    for j in range(2):
                        b4 = (step + j) % GB
                        b = g * GB + b4
                        nc.tensor.matmul(
                            a_ps[j][32 * b4:32 * (b4 + 1), :],
                            p_sb[k][:, 32 * b4:32 * (b4 + 1)],
                            xb[b][:, k * T + j * 512:k * T + j * 512 + 512],
                            start=(k == 0), stop=(k == KC - 1),
                            tile_position=(0, 32 * b4),
                        )
            for j in range(2):
                if first:
                    nc.scalar.copy(lg[:, j * 512:(j + 1) * 512], a_ps[j][:])
                else:
                    nc.vector.tensor_add(
                        lg[:, j * 512:(j + 1) * 512],
                        lg[:, j * 512:(j + 1) * 512], a_ps[j][:],
                    )

        # --- iteration 0 (uniform c = 1/N via constant lhsT) ---
        # groups staggered one phase apart so g1's PE phases fill g0's
        # serial (DVE/ACT) chains
        preload(AF.Sqrt)
        sp0 = q_phase(0, cn[:], 0)
        sp1 = q_phase(1, cn[:], 0)
        vs = {0: extract_squash(sp0)}
        update(0, vs[0], first=True)
        vs[1] = extract_squash(sp1)
        preload(AF.Exp, dep=vs[1][:, 0:1])
        update(1, vs[1], first=True)

        # --- iterations 1, 2 ---
        for it in (1, 2):
            cts0 = softmax_transpose(0)
            sp0 = q_phase(0, cts0[:], 128)
            cts1 = softmax_transpose(1)
            preload(AF.Sqrt, dep=ets[1][:, 0:1])
            vs[0] = extract_squash(sp0)
            sp1 = q_phase(1, cts1[:], 128)
            if it == 1:
                update(0, vs[0], first=False)
                vs[1] = extract_squash(sp1)
                preload(AF.Exp, dep=vs[1][:, 0:1])
                update(1, vs[1], first=False)
            else:
                nc.sync.dma_start(out_d[0:GB], vs[0][:])
                vs[1] = extract_squash(sp1)
                nc.sync.dma_start(out_d[GB:2 * GB], vs[1][:])


_NC_CACHE = {}


def _get_nc():
    if "nc" not in _NC_CACHE:
        _NC_CACHE["nc"] = _build_bass()
    return _NC_CACHE["nc"]


def _make_in_maps(x, W):
    BFnp = ml_dtypes.bfloat16
    x = np.asarray(x, np.float32)
    W = np.asarray(W, np.float32)
    w_bf = np.ascontiguousarray(W.reshape(KC, 128, O)).astype(BFnp)
    wt = np.ascontiguousarray(W.reshape(C, OG, 128).transpose(1, 2, 0)).astype(BFnp)
    e16 = (np.arange(128)[None, :] % D == np.arange(D)[:, None]).astype(BFnp)
    oo = np.arange(128)
    bn = np.arange(128)
    bm = np.stack(
        [((g * 8 + oo[:, None] // D) == (bn[None, :] % N)) for g in range(OG)]
    ).astype(BFnp)
    dm4 = ((np.arange(O)[None, :] // D) == (bn[:, None] % N)).astype(np.float32)
    # [id128 | on4]: on4[bn, j] = (bn // N == j)
    idon = np.zeros((128, 128 + GB), np.float32)
    idon[:, :128] = np.eye(128)
    idon[bn, 128 + bn // N] = 1.0
    idon = idon.astype(BFnp)
    id128f = np.eye(128, dtype=np.float32)
    cn = np.full((128, 128), 1.0 / N, BFnp)

    in_maps = []
    for core in range(NCORES):
        xs = x[core * BPC:(core + 1) * BPC]              # (8, C, T)
        # (b, c, t) -> [b, 128, (k, t)]
        xbt = np.ascontiguousarray(
            xs.reshape(BPC, KC, 128, T).transpose(0, 2, 1, 3).reshape(
                BPC, 128, KC * T)
        ).astype(BFnp)
        # transposed layout: [g, 128t, (m, b4, c)]
        xt4 = np.zeros((NG, 128, MT * GB * C), BFnp)
        for g in range(NG):
            for b4 in range(GB):
                xtb = xs[g * GB + b4].T                  # (T, C) f32
                blocks = xtb.reshape(MT, 128, C).astype(BFnp)  # (m, tl, c)
                for m in range(MT):
                    xt4[g, :, m * GB * C + b4 * C:(m * GB + b4 + 1) * C] = blocks[m]
        in_maps.append(
            {
                "xb": xbt, "xt": xt4, "wsb": w_bf, "wt": wt, "e16": e16,
                "bm": bm, "dm4": dm4, "idon": idon, "id128f": id128f,
                "cn": cn,
            }
        )
    return in_maps


def run(x, W, trace=False):
    in_maps = _make_in_maps(x, W)
    nc = _get_nc()
    res = run_bass_kernel_spmd(nc, in_maps, core_ids=list(range(NCORES)), trace=trace)
    out = np.concatenate([r["out"] for r in res.results], axis=0)
    return out, res


def kernel(x, W, out_num_capsule=N, out_dim_capsule=D, routings=3, **_):
    out, _res = run(x, W, trace=False)
    return out


# revision 24
# speedup vs baseline: 1.1392x; 1.0274x over previous
"""Capsule routing kernel v3 (Conv1D k=1 -> dynamic routing) for TRN2, 8 cores.

Data-parallel over batch (8 batches/core), 2 groups of 4 batches stacked on
the 128-partition dim as (b,n).  u_hat is never materialized; routing is
factorized through x:
    s[n,d] = sum_c Q[n,c] W[c,nD+d],  Q = c @ x^T        (PE)
    b[n,t] += sum_c P[c,n] x[c,t],    P = W . vmask      (PE)

v3 changes vs v2 (trace-driven):
  * All transposes via PE identity-matmul (lhsT=chunk, rhs=id128) instead of
    serial DMA XBAR transposes (48.6us -> ~4us).  The softmax partition-sums
    ride the same matmul: rhs = [id128 | on4] gives etT and Z^T in one pass.
  * 1/Z via reciprocal_approx_fast on t-major [128,4] chunks (27us -> ~1us).
  * Softmax normalize fused into the transpose psum->sbuf copy as a
    stride-0-broadcast tensor_mul.
  * Iteration 0 (uniform c) via the same Q matmul path with a constant-1/N
    lhsT (kills 18us of DVE reduce_sum, warms the PE during the x DMA).
  * Q matmul col-tiled: 4 concurrent 32-wide strips (tile_position), rhs
    FD=256 per batch; psum comes out already in q_sb layout (no extraction).
  * x loaded as 8+2 big DMAs split across sync and scalar HWDGE queues.
"""

import contextlib

import numpy as np
import ml_dtypes

import concourse.bass as bass
import concourse.tile as tile
from concourse import bacc, mybir
from concourse.bass_utils import run_bass_kernel_spmd

F32 = mybir.dt.float32
BF16 = mybir.dt.bfloat16
AF = mybir.ActivationFunctionType
AX = mybir.AxisListType
ALU = mybir.AluOpType

B, C, T = 64, 256, 1024
N, D = 32, 16
O = N * D            # 512
NCORES = 8
BPC = B // NCORES    # 8 batches per core
NG = 2               # groups per core
GB = 4               # batches per group (stacked as (b,n) on 128 partitions)
KC = C // 128        # 2 contraction chunks
MT = T // 128        # 8 t-chunks
OG = O // 128        # 4 o-chunks
EPS = 1e-7


def _build_bass():
    nc = bacc.Bacc(
        "TRN2",
        target_bir_lowering=False,
        debug=False,
        enable_asserts=False,
        num_devices=NCORES,
    )
    # x in (c,t) layout: per batch one [128, KC*T] tile (cols = (k,t))
    xb_d = nc.dram_tensor("xb", [BPC, 128, KC * T], BF16, kind="ExternalInput").ap()
    # x transposed: per group one [128, MT*GB*C] tile (cols = (m, b4, c))
    xt_d = nc.dram_tensor("xt", [NG, 128, MT * GB * C], BF16, kind="ExternalInput").ap()
    w_d = nc.dram_tensor("wsb", [KC, 128, O], BF16, kind="ExternalInput").ap()
    wt_d = nc.dram_tensor("wt", [OG, 128, C], BF16, kind="ExternalInput").ap()
    e16_d = nc.dram_tensor("e16", [D, 128], BF16, kind="ExternalInput").ap()
    bm_d = nc.dram_tensor("bm", [OG, 128, 128], BF16, kind="ExternalInput").ap()
    dm4_d = nc.dram_tensor("dm4", [128, O], F32, kind="ExternalInput").ap()
    # identity+ones combined rhs for transpose matmuls: [128, 132]
    idon_d = nc.dram_tensor("idon", [128, 128 + GB], BF16, kind="ExternalInput").ap()
    id128f_d = nc.dram_tensor("id128f", [128, 128], F32, kind="ExternalInput").ap()
    cn_d = nc.dram_tensor("cn", [128, 128], BF16, kind="ExternalInput").ap()
    out_d = nc.dram_tensor("out", [BPC, N, D], F32, kind="ExternalOutput").ap()

    with tile.TileContext(nc) as tc:
        _kernel_body(tc, out_d, xb_d, xt_d, w_d, wt_d, e16_d, bm_d, dm4_d,
                     idon_d, id128f_d, cn_d)
    nc.compile()
    return nc


def _kernel_body(tc, out_d, xb_d, xt_d, w_d, wt_d, e16_d, bm_d, dm4_d,
                 idon_d, id128f_d, cn_d):
    nc = tc.nc
    ctx = contextlib.ExitStack()
    with ctx:
        const = ctx.enter_context(tc.tile_pool(name="const", bufs=1))
        xbp = ctx.enter_context(tc.tile_pool(name="xbp", bufs=BPC))
        xtp = ctx.enter_context(tc.tile_pool(name="xtp", bufs=2 * NG))
        lgp = ctx.enter_context(tc.tile_pool(name="lgp", bufs=NG))
        etp = ctx.enter_context(tc.tile_pool(name="etp", bufs=2))
        ctsp = ctx.enter_context(tc.tile_pool(name="ctsp", bufs=2))
        rzp = ctx.enter_context(tc.tile_pool(name="rzp", bufs=2))
        qsp = ctx.enter_context(tc.tile_pool(name="qsp", bufs=2))
        qtp = ctx.enter_context(tc.tile_pool(name="qtp", bufs=4))
        pbp = ctx.enter_context(tc.tile_pool(name="pbp", bufs=4))
        vtp = ctx.enter_context(tc.tile_pool(name="vtp", bufs=2))
        vbp = ctx.enter_context(tc.tile_pool(name="vbp", bufs=2))
        vmp = ctx.enter_context(tc.tile_pool(name="vmp", bufs=8))
        smp = ctx.enter_context(tc.tile_pool(name="smp", bufs=2))
        vp = ctx.enter_context(tc.tile_pool(name="vp", bufs=4))
        tinyp = ctx.enter_context(tc.tile_pool(name="tinyp", bufs=8))
        # PSUM pools
        ptp = ctx.enter_context(tc.tile_pool(name="ptp", bufs=2, space="PSUM"))
        pqp = ctx.enter_context(tc.tile_pool(name="pqp", bufs=2, space="PSUM"))
        pband = ctx.enter_context(tc.tile_pool(name="pband", bufs=4, space="PSUM"))

        # --- loads.  iter-0-critical tensors first (cn, idon, xt g0) so
        # their DMA-completion semaphore targets are small and the first Q
        # matmuls can start as soon as those transfers land. ---
        cn = const.tile([128, 128], BF16, name="cn", tag="cn")
        nc.sync.dma_start(cn[:], cn_d[:])
        idon = const.tile([128, 128 + GB], BF16, name="idon", tag="idon")
        nc.sync.dma_start(idon[:], idon_d[:])
        HM = MT // 2
        xth = {}
        for g in range(NG):
            for h in range(2):
                t = xtp.tile([128, HM * GB * C], BF16, name="xt", tag="xt")
                nc.scalar.dma_start(
                    t[:], xt_d[g][:, h * HM * GB * C:(h + 1) * HM * GB * C])
                xth[g, h] = t
        w_sb = [const.tile([128, O], BF16, name=f"w{k}", tag=f"w{k}") for k in range(KC)]
        for k in range(KC):
            nc.sync.dma_start(w_sb[k][:], w_d[k])
        dm4 = const.tile([128, O], F32, name="dm4", tag="dm4")
        nc.sync.dma_start(dm4[:], dm4_d[:])
        wt_sb = [const.tile([128, C], BF16, name=f"wt{g}", tag=f"wt{g}") for g in range(OG)]
        for g in range(OG):
            nc.sync.dma_start(wt_sb[g][:], wt_d[g])
        e16 = const.tile([D, 128], BF16, name="e16", tag="e16")
        nc.sync.dma_start(e16[:], e16_d[:])
        bm = [const.tile([128, 128], BF16, name=f"bm{g}", tag=f"bm{g}") for g in range(OG)]
        for g in range(OG):
            nc.sync.dma_start(bm[g][:], bm_d[g])
        id128f = const.tile([128, 128], F32, name="id128f", tag="id128f")
        nc.sync.dma_start(id128f[:], id128f_d[:])
        xb = {}
        for b in range(BPC):
            t = xbp.tile([128, KC * T], BF16, name="xb", tag="xb")
            nc.sync.dma_start(t[:], xb_d[b])
            xb[b] = t

        logits = {g: lgp.tile([128, T], F32, name="lg", tag="lg") for g in range(NG)}
        ets = {}

        # scratch for activation-table preloading (keeps the ~1.3us
        # ACT_TABLE_LOAD for Exp<->Sqrt switches off the critical path)
        scr = const.tile([128, 1], F32, name="scr", tag="scr")
        scr2 = const.tile([128, 1], F32, name="scr2", tag="scr2")
        nc.gpsimd.memset(scr[:], 1.0)

        def preload(func, dep=None):
            nc.scalar.activation(scr2[:], scr[:] if dep is None else dep, func)

        def softmax_transpose(g):
            """logits [bn, T] -> cts [128t, (m, bn)] bf16, normalized.

            Per 128-t chunk: one PE matmul with lhsT = exp-chunk and
            rhs = [id128 | on4] yields etT (cols 0:128) and Z^T (cols 128:132)
            in psum; 1/Z via fast reciprocal; normalize fused into the
            psum->sbuf copy as a broadcast multiply.
            """
            lg = logits[g]
            et = etp.tile([128, T], BF16, name="et", tag="et")
            ets[g] = et
            for jh in range(2):
                nc.scalar.activation(
                    et[:, jh * 512:(jh + 1) * 512],
                    lg[:, jh * 512:(jh + 1) * 512], AF.Exp)
            cts = ctsp.tile([128, MT * 128], BF16, name="cts", tag="cts")
            rzt = rzp.tile([128, MT * GB], F32, name="rzt", tag="rzt")
            for m in range(MT):
                tp = ptp.tile([128, 512], F32, name="tp", tag="tp")
                nc.tensor.matmul(
                    tp[:, 0:128 + GB], et[:, m * 128:(m + 1) * 128], idon[:],
                    start=True, stop=True,
                )
                nc.vector.reciprocal(
                    rzt[:, m * GB:(m + 1) * GB], tp[:, 128:128 + GB],
                )
                # cts[:, m-chunk] = etT * (1/Z) broadcast over the 32 n-cols
                nc.vector.tensor_tensor(
                    cts[:, m * 128:(m + 1) * 128].rearrange(
                        "p (b n) -> p b n", n=N),
                    tp[:, 0:128].rearrange("p (b n) -> p b n", n=N),
                    rzt[:, m * GB:(m + 1) * GB].unsqueeze(2).broadcast_to(
                        [128, GB, N]),
                    ALU.mult,
                )
            return cts

        def q_phase(g, cts_ap, cstride):
            """Q[bn, c] col-tiled: strip b4 <- cts chunk-strip ^T @ xt4-slice.

            cts_ap: SBUF AP base; strip (m, b4) slice is
            cts_ap[:, m*cstride + 32*b4 : m*cstride + 32*b4 + 32]
            (cstride=0 for the constant iteration-0 lhsT).
            """
            # Two psum banks: strips {0,1} in qA, {2,3} in qB.  One open
            # accumulation group per bank; pairs (0,2) then (1,3) issue
            # back-to-back at different PE col-groups -> concurrent.
            qA = pqp.tile([128, 512], F32, name="qA", tag="q_ps")
            qB = pqp.tile([128, 512], F32, name="qB", tag="q_ps")
            for phase in range(2):
                for m in range(MT):
                    for b4 in (phase, phase + 2):
                        q_ps = qA if b4 < 2 else qB
                        off = m * cstride + 32 * b4
                        xt_t = xth[g, m // HM]
                        mo = (m % HM) * GB * C + b4 * C
                        nc.tensor.matmul(
                            q_ps[32 * b4:32 * (b4 + 1), 0:C],
                            cts_ap[:, off:off + 32],
                            xt_t[:, mo:mo + C],
                            start=(m == 0), stop=(m == MT - 1),
                            tile_position=(0, 32 * b4),
                        )
            q_sb = qsp.tile([128, C], BF16, name="q_sb", tag="q_sb")
            nc.scalar.copy(q_sb[0:64, :], qA[0:64, 0:C])
            nc.scalar.copy(q_sb[64:128, :], qB[64:128, 0:C])
            # transpose Q via identity matmul, then s = Qt^T @ W
            s_ps = pband.tile([128, O], F32, name="s_ps", tag="band")
            qts = []
            for h in range(KC):
                qt_ps = pqp.tile([128, 512], F32, name="qt_ps", tag="q_ps")
                nc.tensor.matmul(
                    qt_ps[:, 0:128], q_sb[:, h * 128:(h + 1) * 128], idon[:, 0:128],
                    start=True, stop=True,
                )
                qt = qtp.tile([128, 128], BF16, name="qt", tag="qt")
                nc.scalar.copy(qt[:], qt_ps[:, 0:128])
                qts.append(qt)
            for h in range(KC):
                nc.tensor.matmul(
                    s_ps[:], qts[h][:], w_sb[h][:],
                    start=(h == 0), stop=(h == KC - 1),
                )
            return s_ps

        def extract_squash(s_ps):
            """psum s_full (128(b,n), O) -> v (128, D) f32 via mask+strided reduce."""
            sm = smp.tile([128, O], F32, name="sm", tag="sm")
            nc.vector.tensor_mul(sm[:], s_ps[:], dm4[:])
            s_t = tinyp.tile([128, D], F32, name="s_t", tag="s_t")
            nc.vector.reduce_sum(
                s_t[:], sm[:].rearrange("p (n d) -> p d n", d=D), axis=AX.X
            )
            sq = tinyp.tile([128, D], F32, name="sq", tag="sq")
            nc.vector.tensor_mul(sq[:], s_t[:], s_t[:])
            s2 = tinyp.tile([128, 1], F32, name="s2", tag="s2")
            nc.vector.reduce_sum(s2[:], sq[:], axis=AX.X)
            s2e = tinyp.tile([128, 1], F32, name="s2e", tag="s2e")
            nc.vector.tensor_scalar_add(s2e[:], s2[:], EPS)
            rt = tinyp.tile([128, 1], F32, name="rt", tag="rt")
            nc.scalar.sqrt(rt[:], s2e[:])
            d1 = tinyp.tile([128, 1], F32, name="d1", tag="d1")
            nc.vector.tensor_scalar_add(d1[:], s2e[:], 1.0)
            r1 = tinyp.tile([128, 1], F32, name="r1", tag="r1")
            nc.vector.reciprocal(r1[:], d1[:])
            sc = tinyp.tile([128, 1], F32, name="sc", tag="sc")
            nc.vector.tensor_mul(sc[:], rt[:], r1[:])
            v = vp.tile([128, D], F32, name="v", tag="v")
            nc.vector.tensor_scalar_mul(v[:], s_t[:], sc[:])
            return v

        def update(g, v, first):
            """logits ((b,n), t) += x^T (W . vmask) for the 4 stacked batches."""
            vt_ps = pqp.tile([128, 512], F32, name="vt_ps", tag="q_ps")
            nc.tensor.transpose(vt_ps[0:D, 0:128], v[:], id128f[:])
            vt_bf = vtp.tile([D, 128], BF16, name="vt_bf", tag="vt_bf")
            nc.vector.tensor_copy(vt_bf[:], vt_ps[0:D, 0:128])
            vbc_ps = pqp.tile([128, 512], F32, name="vbc", tag="q_ps")
            nc.tensor.matmul(vbc_ps[:, 0:128], e16[:], vt_bf[:], start=True, stop=True)
            vbc_sb = vbp.tile([128, 128], BF16, name="vbc_sb", tag="vbc_sb")
            nc.scalar.copy(vbc_sb[:], vbc_ps[:, 0:128])
            vms = []
            for g4 in range(OG):
                vm = vmp.tile([128, 128], BF16, name="vm", tag="vm")
                nc.vector.tensor_mul(vm[:], vbc_sb[:], bm[g4][:])
                vms.append(vm)
            p_sb = []
            for h in range(KC):
                p_ps = ptp.tile([128, 512], F32, name="p_ps", tag="tp")
                for g4 in range(OG):
                    nc.tensor.matmul(
                        p_ps[:, 0:128], wt_sb[g4][:, h * 128:(h + 1) * 128], vms[g4][:],
                        start=(g4 == 0), stop=(g4 == OG - 1),
                    )
                pb = pbp.tile([128, 128], BF16, name="pb", tag="pb")
                nc.scalar.copy(pb[:], p_ps[:, 0:128])
                p_sb.append(pb)
            lg = logits[g]
            # two banks (t-halves); strip pairing offset by 1 so concurrent
            # MMs land on different PE col-groups
            a_ps = [
                pband.tile([128, 512], F32, name=f"a_ps{j}", tag="band")
                for j in range(2)
            ]
            for step in range(GB):
                for k in range(KC):
                    for j in range(2):
                        b4 = (step + j) % GB
                        b = g * GB + b4
                        nc.tensor.matmul(
                            a_ps[j][32 * b4:32 * (b4 + 1), :],
                            p_sb[k][:, 32 * b4:32 * (b4 + 1)],
                            xb[b][:, k * T + j * 512:k * T + j * 512 + 512],
                            start=(k == 0), stop=(k == KC - 1),
                            tile_position=(0, 32 * b4),
                        )
            for j in range(2):
                if first:
                    nc.scalar.copy(lg[:, j * 512:(j + 1) * 512], a_ps[j][:])
                else:
                    nc.vector.tensor_add(
                        lg[:, j * 512:(j + 1) * 512],
                        lg[:, j * 512:(j + 1) * 512], a_ps[j][:],
                    )

        # --- iteration 0 (uniform c = 1/N via constant lhsT) ---
        # groups staggered one phase apart so g1's PE phases fill g0's
        # serial (DVE/ACT) chains
        preload(AF.Sqrt)
        sp0 = q_phase(0, cn[:], 0)
        sp1 = q_phase(1, cn[:], 0)
        vs = {0: extract_squash(sp0)}
        update(0, vs[0], first=True)
        vs[1] = extract_squash(sp1)
        preload(AF.Exp, dep=vs[1][:, 0:1])
        update(1, vs[1], first=True)

        # --- iterations 1, 2 ---
        for it in (1, 2):
            cts0 = softmax_transpose(0)
            sp0 = q_phase(0, cts0[:], 128)
            cts1 = softmax_transpose(1)
            preload(AF.Sqrt, dep=ets[1][:, 0:1])
            vs[0] = extract_squash(sp0)
            sp1 = q_phase(1, cts1[:], 128)
            if it == 1:
                update(0, vs[0], first=False)
                vs[1] = extract_squash(sp1)
                preload(AF.Exp, dep=vs[1][:, 0:1])
                update(1, vs[1], first=False)
            else:
                nc.sync.dma_start(out_d[0:GB], vs[0][:])
                vs[1] = extract_squash(sp1)
                nc.sync.dma_start(out_d[GB:2 * GB], vs[1][:])


_NC_CACHE = {}


def _get_nc():
    if "nc" not in _NC_CACHE:
        _NC_CACHE["nc"] = _build_bass()
    return _NC_CACHE["nc"]


def _make_in_maps(x, W):
    BFnp = ml_dtypes.bfloat16
    x = np.asarray(x, np.float32)
    W = np.asarray(W, np.float32)
    w_bf = np.ascontiguousarray(W.reshape(KC, 128, O)).astype(BFnp)
    wt = np.ascontiguousarray(W.reshape(C, OG, 128).transpose(1, 2, 0)).astype(BFnp)
    e16 = (np.arange(128)[None, :] % D == np.arange(D)[:, None]).astype(BFnp)
    oo = np.arange(128)
    bn = np.arange(128)
    bm = np.stack(
        [((g * 8 + oo[:, None] // D) == (bn[None, :] % N)) for g in range(OG)]
    ).astype(BFnp)
    dm4 = ((np.arange(O)[None, :] // D) == (bn[:, None] % N)).astype(np.float32)
    # [id128 | on4]: on4[bn, j] = (bn // N == j)
    idon = np.zeros((128, 128 + GB), np.float32)
    idon[:, :128] = np.eye(128)
    idon[bn, 128 + bn // N] = 1.0
    idon = idon.astype(BFnp)
    id128f = np.eye(128, dtype=np.float32)
    cn = np.full((128, 128), 1.0 / N, BFnp)

    in_maps = []
    for core in range(NCORES):
        xs = x[core * BPC:(core + 1) * BPC]              # (8, C, T)
        # (b, c, t) -> [b, 128, (k, t)]
        xbt = np.ascontiguousarray(
            xs.reshape(BPC, KC, 128, T).transpose(0, 2, 1, 3).reshape(
                BPC, 128, KC * T)
        ).astype(BFnp)
        # transposed layout: [g, 128t, (m, b4, c)]
        xt4 = np.zeros((NG, 128, MT * GB * C), BFnp)
        for g in range(NG):
            for b4 in range(GB):
                xtb = xs[g * GB + b4].T                  # (T, C) f32
                blocks = xtb.reshape(MT, 128, C).astype(BFnp)  # (m, tl, c)
                for m in range(MT):
                    xt4[g, :, m * GB * C + b4 * C:(m * GB + b4 + 1) * C] = blocks[m]
        in_maps.append(
            {
                "xb": xbt, "xt": xt4, "wsb": w_bf, "wt": wt, "e16": e16,
                "bm": bm, "dm4": dm4, "idon": idon, "id128f": id128f,
                "cn": cn,
            }
        )
    return in_maps


def run(x, W, trace=False):
    in_maps = _make_in_maps(x, W)
    nc = _get_nc()
    res = run_bass_kernel_spmd(nc, in_maps, core_ids=list(range(NCORES)), trace=trace)
    out = np.concatenate([r["out"] for r in res.results], axis=0)
    return out, res


def kernel(x, W, out_num_capsule=N, out_dim_capsule=D, routings=3, **_):
    out, _res = run(x, W, trace=False)
    return out
